# revision 3
# baseline (speedup 1.0000x reference)
# GAT (graph attention) layer on 8 Trainium2 NeuronCores.
#
# Strategy: target-sharded edges. Each core owns 1/8 of the target nodes and
# processes exactly the edges pointing into its range, so the segment-sum
# (softmax denominator + weighted feature aggregation) is core-local. The only
# collective is an AllReduce-max of one scalar (the global max attention
# score, needed to reproduce the reference's `exp(e - e.max())` + `+1e-16`
# epsilon numerics exactly).
#
# Per core:
#   Phase T: proj = x @ W and s_src per node -> gather tables A/B (bf16 rows
#     [proj(128) | s_hi(4) | s_lo(4) | pad], 512B, split at node 25000 so
#     dma_gather's int16 indices stay in range), plus a local s_trg table.
#   Phase E: edges host-sorted by target into windows of 128 target nodes,
#     each window's edges split-sorted by src half. Per window: two ucode
#     dma_gathers fetch [proj|s_src] rows; s_trg comes from a one-hot matmul
#     (host-provided selT) against the local s_trg window slice; then
#     e = leaky_relu(s_src + s_trg), ex = exp(e - 24), and two
#     PSUM-accumulated matmuls against a one-hot selection matrix (is_equal
#     vs an iota tile) give per-window weighted sums and denominators.
#     A running max of raw scores is kept on the side.
#   Collective: AllReduce(max) -> global M.
#   Phase F: out = elu(W/(D + 1e-16*exp(M-24)) + x + bias)  (identical to the
#     reference's shifted softmax + epsilon, since all sums carry exp(-24)).
import sys
from contextlib import ExitStack

import numpy as np

sys.path.insert(0, "/opt/trn_rl_repo")

import ml_dtypes  # noqa: E402

import concourse.bass as bass  # noqa: E402,F401
import concourse.mybir as mybir  # noqa: E402
import concourse.tile as tile  # noqa: E402
from concourse import bacc  # noqa: E402
from concourse.masks import make_identity  # noqa: E402

P = 128
NH, FOUT = 4, 32
NHF = NH * FOUT  # 128
FIN = 128
ROW = 2 * P  # gather-table row: 256 bf16 = 512B
LEAKY = 0.2
SHIFT = 24.0
F32 = mybir.dt.float32
BF16 = mybir.dt.bfloat16
I16 = mybir.dt.int16
AX = mybir.AxisListType
OP = mybir.AluOpType
ACT = mybir.ActivationFunctionType
BF = ml_dtypes.bfloat16


def _wrap16(flat):
    """[..., L] -> dma_gather layout [..., 16, L//16] replicated to 128 rows."""
    L = flat.shape[-1]
    w = flat.reshape(flat.shape[:-1] + (L // 16, 16))
    w = np.swapaxes(w, -1, -2)  # [..., 16, L//16]
    return np.tile(w, (1, 1, 8, 1)).reshape(flat.shape[:-1] + (P, L // 16))


def _prepare_edges(edge_index, n_nodes, n_cores):
    npc = n_nodes // n_cores
    nw = (npc + P - 1) // P
    half = n_nodes // 2
    src = np.ascontiguousarray(edge_index[0]).astype(np.int64)
    trg = np.ascontiguousarray(edge_index[1]).astype(np.int64)
    E = src.shape[0]
    wglob = (trg // npc) * nw + (trg % npc) // P
    isb = (src >= half).astype(np.int64)
    order = np.argsort(wglob * 2 + isb, kind="stable")
    src_s, trg_s, wg_s, isb_s = src[order], trg[order], wglob[order], isb[order]
    nwin = n_cores * nw
    cnt_a = np.bincount(wg_s * 2 + isb_s, minlength=2 * nwin)[0::2]
    cnt_b = np.bincount(wg_s * 2 + isb_s, minlength=2 * nwin)[1::2]
    t_a = max(1, int(np.ceil(cnt_a.max() / P)))
    t_b = max(1, int(np.ceil(cnt_b.max() / P)))
    t_eff = t_a + t_b
    # position within (window, half) group
    gkey = wg_s * 2 + isb_s
    gstart = np.concatenate([[0], np.cumsum(np.bincount(gkey, minlength=2 * nwin))])[:-1]
    jj = np.arange(E) - gstart[gkey]
    t_loc = jj // P
    p_idx = jj % P
    t_idx = np.where(isb_s == 1, t_a + t_loc, t_loc)
    c = wg_s // nw
    wloc = wg_s % nw
    rel = (trg_s % npc) - wloc * P  # 0..127

    idx_a = np.zeros((n_cores, nw, t_a * P), np.int16)
    idx_b = np.zeros((n_cores, nw, t_b * P), np.int16)
    ma = isb_s == 0
    idx_a[c[ma], wloc[ma], t_loc[ma] * P + p_idx[ma]] = src_s[ma].astype(np.int16)
    mb = isb_s == 1
    idx_b[c[mb], wloc[mb], t_loc[mb] * P + p_idx[mb]] = (src_s[mb] - half).astype(np.int16)

    rel_arr = np.full((n_cores, nw * P, t_eff), -1.0, np.float32)
    rel_arr[c, wloc * P + p_idx, t_idx] = rel
    selt = np.zeros((n_cores, nw * P, t_eff * P), BF)
    selt[c, wloc * P + rel, t_idx * P + p_idx] = 1.0

    ia = _wrap16(idx_a)  # [nc, nw, 128, t_a*8]
    ib = _wrap16(idx_b)
    gidx = np.concatenate([ia, ib], axis=-1).reshape(n_cores, nw * P, t_eff * 8)
    return t_a, t_b, gidx, rel_arr.astype(BF), selt


def build_bass(n_nodes, n_cores, t_a, t_b, debug_out=False, sim_no_cc=False):
    npc = n_nodes // n_cores
    nw = (npc + P - 1) // P
    nt = (n_nodes + P - 1) // P
    half = n_nodes // 2
    t_eff = t_a + t_b
    nc = bacc.Bacc("TRN2", target_bir_lowering=False, debug=False,
                   num_devices=n_cores)

    x = nc.dram_tensor("x", [n_nodes, FIN], F32, kind="ExternalInput")
    xloc = nc.dram_tensor("xloc", [npc, FIN], F32, kind="ExternalInput")
    w_in = nc.dram_tensor("W", [FIN, NHF], F32, kind="ExternalInput")
    amat = nc.dram_tensor("amat", [NHF, 2 * NH], F32, kind="ExternalInput")
    bias_in = nc.dram_tensor("bias", [1, NHF], F32, kind="ExternalInput")
    gidx = nc.dram_tensor("gidx", [nw * P, t_eff * 8], I16, kind="ExternalInput")
    trg_rel = nc.dram_tensor("trg_rel", [nw * P, t_eff], BF16,
                             kind="ExternalInput")
    selt_in = nc.dram_tensor("selt", [nw * P, t_eff * P], BF16,
                             kind="ExternalInput")
    out = nc.dram_tensor("out", [npc, NHF], F32, kind="ExternalOutput")

    dbgk = "ExternalOutput" if debug_out else "Internal"
    dbg = nc.dram_tensor("dbg", [8, P], F32, kind="ExternalOutput") if debug_out else None
    tab_a = nc.dram_tensor("tab_a", [half, ROW], BF16)
    tab_b = nc.dram_tensor("tab_b", [n_nodes - half, ROW], BF16)
    tab_c = nc.dram_tensor("tab_c", [nw * P, 2 * NH], BF16, kind=dbgk)
    acc_wt = nc.dram_tensor("acc_wt", [nw * P, P], F32, kind=dbgk)
    acc_d = nc.dram_tensor("acc_d", [nw * NH, P], F32, kind=dbgk)

    with tile.TileContext(nc) as tc, ExitStack() as ctx:
        const = ctx.enter_context(tc.tile_pool(name="const", bufs=1))
        sb = ctx.enter_context(tc.tile_pool(name="sb", bufs=3))
        sbg = ctx.enter_context(tc.tile_pool(name="sbg", bufs=2))
        dram = ctx.enter_context(tc.tile_pool(name="dram", bufs=1, space="DRAM"))

        ident = const.tile([P, P], F32)
        make_identity(nc, ident[:])
        c_i32 = const.tile([P, P], mybir.dt.int32)
        nc.gpsimd.iota(c_i32[:], pattern=[[1, P]], base=0, channel_multiplier=0)
        c_bf = const.tile([P, P], BF16)
        nc.vector.tensor_copy(c_bf[:], c_i32[:])

        sb_w = const.tile([FIN, NHF], F32)
        nc.sync.dma_start(sb_w[:], w_in[:])
        sb_a = const.tile([NHF, 2 * NH], F32)
        nc.sync.dma_start(sb_a[:], amat[:])
        sb_bias = const.tile([1, NHF], F32)
        nc.sync.dma_start(sb_bias[:], bias_in[:])

        with tc.tile_pool(name="ps0", bufs=1, space="PSUM") as ps0:
            ps_wt = ps0.tile([NHF, FIN], F32, tag="pst")
            nc.tensor.transpose(ps_wt[:], sb_w[:], ident[:])
            sb_wt = sb.tile([NHF, FIN], F32)
            nc.vector.tensor_copy(sb_wt[:], ps_wt[:])
            ps_wa = ps0.tile([FIN, 2 * NH], F32, tag="pst2")
            nc.tensor.matmul(ps_wa[:], lhsT=sb_wt[:], rhs=sb_a[:], start=True,
                             stop=True)
            wcat = const.tile([FIN, NHF + 2 * NH], F32)
            nc.vector.tensor_copy(wcat[:, 0:NHF], sb_w[:])
            nc.vector.tensor_copy(wcat[:, NHF:NHF + 2 * NH], ps_wa[:])
            ones_row = const.tile([1, P], F32)
            nc.gpsimd.memset(ones_row[:], 1.0)
            ps_b = ps0.tile([P, NHF], F32, tag="pst3")
            nc.tensor.matmul(ps_b[:], lhsT=ones_row[:], rhs=sb_bias[:],
                             start=True, stop=True)
            sb_b = const.tile([P, NHF], F32)
            nc.vector.tensor_copy(sb_b[:], ps_b[:])

        bias_zero = const.tile([P, 1], F32)
        nc.gpsimd.memset(bias_zero[:], 0.0)
        bias_mshift = const.tile([P, 1], F32)
        nc.gpsimd.memset(bias_mshift[:], -SHIFT)

        # --- phase T: gather tables (global) + local s_trg table ---
        with tc.tile_pool(name="psT", bufs=2, space="PSUM") as psT:
            for i in range(nt):
                r0 = i * P
                rows = min(P, n_nodes - r0)
                xt = sb.tile([P, FIN], F32, tag="xt")
                nc.sync.dma_start(xt[:rows], x[r0:r0 + rows, :])
                ps_xt = psT.tile([P, P], F32, tag="ps_xt")
                nc.tensor.transpose(ps_xt[:, :rows], xt[:rows, :],
                                    ident[:rows, :rows])
                x_tr = sb.tile([P, P], F32, tag="x_tr")
                nc.vector.tensor_copy(x_tr[:, :rows], ps_xt[:, :rows])
                ps_tab = psT.tile([P, NHF + NH], F32, tag="ps_tab")
                nc.tensor.matmul(ps_tab[:rows, :], lhsT=x_tr[:, :rows],
                                 rhs=wcat[:, 0:NHF + NH], start=True, stop=True)
                tabt = sb.tile([P, NHF + 2 * NH], BF16, tag="tabt")
                nc.vector.tensor_copy(tabt[:rows, 0:NHF], ps_tab[:rows, 0:NHF])
                nc.vector.tensor_copy(tabt[:rows, NHF:NHF + NH],
                                      ps_tab[:rows, NHF:NHF + NH])
                s_lo = sb.tile([P, NH], F32, tag="s_lo")
                nc.vector.tensor_tensor(s_lo[:rows], ps_tab[:rows, NHF:NHF + NH],
                                        tabt[:rows, NHF:NHF + NH], OP.subtract)
                nc.vector.tensor_copy(tabt[:rows, NHF + NH:NHF + 2 * NH],
                                      s_lo[:rows])
                # route rows to table A/B (split at `half`)
                if r0 + rows <= half:
                    nc.sync.dma_start(tab_a[r0:r0 + rows, 0:NHF + 2 * NH],
                                      tabt[:rows, :])
                elif r0 >= half:
                    nc.sync.dma_start(tab_b[r0 - half:r0 - half + rows,
                                            0:NHF + 2 * NH], tabt[:rows, :])
                else:
                    k = half - r0
                    nc.sync.dma_start(tab_a[r0:half, 0:NHF + 2 * NH], tabt[:k, :])
                    nc.sync.dma_start(tab_b[0:r0 + rows - half, 0:NHF + 2 * NH],
                                      tabt[k:rows, :])
            # local s_trg table from xloc
            for i in range(nw):
                r0 = i * P
                rows = min(P, npc - r0)
                xt = sb.tile([P, FIN], F32, tag="xt")
                nc.sync.dma_start(xt[:rows], xloc[r0:r0 + rows, :])
                ps_xt = psT.tile([P, P], F32, tag="ps_xt")
                nc.tensor.transpose(ps_xt[:, :rows], xt[:rows, :],
                                    ident[:rows, :rows])
                x_tr = sb.tile([P, P], F32, tag="x_tr")
                nc.vector.tensor_copy(x_tr[:, :rows], ps_xt[:, :rows])
                ps_c = psT.tile([P, NH], F32, tag="ps_c")
                nc.tensor.matmul(ps_c[:rows, :], lhsT=x_tr[:, :rows],
                                 rhs=wcat[:, NHF + NH:NHF + 2 * NH], start=True,
                                 stop=True)
                tabc = sb.tile([P, 2 * NH], BF16, tag="tabc")
                if rows < P:
                    nc.gpsimd.memset(tabc[:], 0.0)
                nc.vector.tensor_copy(tabc[:rows, 0:NH], ps_c[:rows, :])
                c_lo = sb.tile([P, NH], F32, tag="c_lo")
                nc.vector.tensor_tensor(c_lo[:rows], ps_c[:rows, :],
                                        tabc[:rows, 0:NH], OP.subtract)
                nc.vector.tensor_copy(tabc[:rows, NH:2 * NH], c_lo[:rows])
                nc.sync.dma_start(tab_c[r0:r0 + P, :], tabc[:])

        # --- phase E ---
        zmax = const.tile([P, t_eff * NH], F32)
        nc.gpsimd.memset(zmax[:], -1e30)
        psE = ctx.enter_context(tc.tile_pool(name="psE", bufs=2, space="PSUM"))

        for w in range(nw):
            er0 = w * P
            idx_t = sb.tile([P, t_eff * 8], I16, tag="idx_t")
            nc.sync.dma_start(idx_t[:], gidx[er0:er0 + P, :])
            rel = sb.tile([P, t_eff], BF16, tag="rel")
            nc.sync.dma_start(rel[:], trg_rel[er0:er0 + P, :])
            selt = sbg.tile([P, t_eff * P], BF16, tag="selt")
            nc.sync.dma_start(selt[:], selt_in[er0:er0 + P, :])
            stw = sb.tile([P, 2 * NH], BF16, tag="stw")
            nc.sync.dma_start(stw[:], tab_c[er0:er0 + P, :])

            gath = sbg.tile([P, t_eff * ROW], BF16, tag="gath")
            g3 = gath[:].rearrange("p (t c) -> p t c", c=ROW)
            nc.gpsimd.dma_gather(
                out_ap=g3[:, 0:t_a, :], in_ap=tab_a[:],
                idxs_ap=idx_t[:, 0:t_a * 8], num_idxs=t_a * P,
                num_idxs_reg=t_a * P, elem_size=ROW, single_packet=False)
            nc.gpsimd.dma_gather(
                out_ap=g3[:, t_a:t_eff, :], in_ap=tab_b[:],
                idxs_ap=idx_t[:, t_a * 8:], num_idxs=t_b * P,
                num_idxs_reg=t_b * P, elem_size=ROW, single_packet=False)

            # s_trg per edge via one-hot matmul (batched into one PSUM bank)
            ps_st = psE.tile([P, t_eff * 2 * NH], F32, tag="ps_st")
            for t in range(t_eff):
                nc.tensor.matmul(ps_st[:, t * 2 * NH:(t + 1) * 2 * NH],
                                 lhsT=selt[:, t * P:(t + 1) * P], rhs=stw[:],
                                 start=True, stop=True)
            st3 = ps_st[:].rearrange("p (t h) -> p t h", h=2 * NH)

            z = sb.tile([P, t_eff * NH], F32, tag="z")
            z3 = z[:].rearrange("p (t h) -> p t h", h=NH)
            nc.vector.tensor_tensor(z3, g3[:, :, NHF:NHF + NH],
                                    g3[:, :, NHF + NH:NHF + 2 * NH], OP.add)
            nc.vector.tensor_tensor(z3, z3, st3[:, :, 0:NH], OP.add)
            nc.vector.tensor_tensor(z3, z3, st3[:, :, NH:2 * NH], OP.add)
            nc.vector.tensor_tensor(zmax[:], zmax[:], z[:], OP.max)
            zl = sb.tile([P, t_eff * NH], F32, tag="zl")
            nc.vector.tensor_scalar_mul(zl[:], z[:], LEAKY)
            nc.vector.tensor_tensor(zl[:], zl[:], z[:], OP.max)
            ex = sb.tile([P, t_eff * NH], BF16, tag="ex")
            nc.scalar.activation(ex[:], zl[:], ACT.Exp, bias=bias_mshift[:])

            sel = sbg.tile([P, t_eff * P], BF16, tag="sel")
            nc.vector.tensor_tensor(
                sel[:].rearrange("p (t q) -> p t q", q=P),
                rel[:, :, None].to_broadcast([P, t_eff, P]),
                c_bf[:, None, :].to_broadcast([P, t_eff, P]),
                OP.is_equal)
            wgt = sbg.tile([P, t_eff * NHF], BF16, tag="wgt")
            nc.vector.tensor_tensor(
                wgt[:].rearrange("p (t h f) -> p t h f", h=NH, f=FOUT),
                g3[:, :, 0:NHF].rearrange("p t (h f) -> p t h f", f=FOUT),
                ex[:].rearrange("p (t h) -> p t h", h=NH)[:, :, :, None]
                .to_broadcast([P, t_eff, NH, FOUT]),
                OP.mult)

            ps_w = psE.tile([P, P], F32, tag="ps_w")
            ps_d = psE.tile([NH, P], F32, tag="ps_d")
            for t in range(t_eff):
                nc.tensor.matmul(ps_w[:], lhsT=wgt[:, t * NHF:(t + 1) * NHF],
                                 rhs=sel[:, t * P:(t + 1) * P],
                                 start=(t == 0), stop=(t == t_eff - 1))
                nc.tensor.matmul(ps_d[:], lhsT=ex[:, t * NH:(t + 1) * NH],
                                 rhs=sel[:, t * P:(t + 1) * P],
                                 start=(t == 0), stop=(t == t_eff - 1))
            wt_sb = sb.tile([P, P], F32, tag="wt_sb")
            nc.vector.tensor_copy(wt_sb[:], ps_w[:])
            d_sb = sb.tile([NH, P], F32, tag="d_sb")
            nc.vector.tensor_copy(d_sb[:], ps_d[:])
            nc.sync.dma_start(acc_wt[w * P:(w + 1) * P, :], wt_sb[:])
            nc.sync.dma_start(acc_d[w * NH:(w + 1) * NH, :], d_sb[:])

        # --- global max + epsilon scalar ---
        zm1 = sb.tile([P, 1], F32, tag="zm1")
        nc.vector.tensor_reduce(zm1[:], zmax[:], axis=AX.X, op=OP.max)
        zm0 = sb.tile([1, 1], F32, tag="zm0")
        nc.gpsimd.tensor_reduce(zm0[:], zm1[:], axis=AX.C, op=OP.max)
        cc_in = dram.tile([1, 1], F32)
        cc_out = dram.tile([1, 1], F32)
        nc.sync.dma_start(cc_in[:], zm0[:])
        if sim_no_cc:
            nc.sync.dma_start(cc_out[:], cc_in[:])
        else:
            nc.gpsimd.collective_compute(
                "AllReduce", OP.max, replica_groups=[list(range(n_cores))],
                ins=[cc_in.opt()], outs=[cc_out.opt()])
        zg = sb.tile([1, 1], F32, tag="zg")
        nc.sync.dma_start(zg[:], cc_out[:])
        eg = sb.tile([1, 1], F32, tag="eg")
        nc.vector.tensor_scalar_mul(eg[:], zg[:], LEAKY)
        nc.vector.tensor_tensor(eg[:], eg[:], zg[:], OP.max)
        ce = sb.tile([1, 1], F32, tag="ce")
        nc.scalar.activation(ce[:], eg[:], ACT.Exp, bias=bias_mshift[:1])
        nc.vector.tensor_scalar_mul(ce[:], ce[:], 1e-16)
        ceps = const.tile([P, 1], F32)
        nc.gpsimd.partition_broadcast(ceps[:], ce[:])

        if dbg is not None:
            dbg_t = sb.tile([1, P], F32, tag="dbg_t")
            nc.gpsimd.memset(dbg_t[:], 0.0)
            nc.vector.tensor_copy(dbg_t[:, 0:1], zm0[:])
            nc.vector.tensor_copy(dbg_t[:, 1:2], zg[:])
            nc.vector.tensor_copy(dbg_t[:, 2:3], eg[:])
            nc.vector.tensor_copy(dbg_t[:, 3:4], ce[:])
            nc.sync.dma_start(dbg[0:1, :], dbg_t[:])

        # --- phase F ---
        for w in range(nw):
            rows = min(P, npc - w * P)
            wt_l = sb.tile([P, P], F32, tag="wt_l")
            nc.sync.dma_start(wt_l[:], acc_wt[w * P:(w + 1) * P, :])
            dt_l = sb.tile([NH, P], F32, tag="dt_l")
            nc.sync.dma_start(dt_l[:], acc_d[w * NH:(w + 1) * NH, :])
            ps_w2 = psE.tile([P, P], F32, tag="ps_w")
            nc.tensor.transpose(ps_w2[:], wt_l[:], ident[:])
            ps_d2 = psE.tile([P, NH], F32, tag="ps_d")
            nc.tensor.transpose(ps_d2[:], dt_l[:], ident[:NH, :NH])
            den = sb.tile([P, NH], F32, tag="den")
            nc.vector.tensor_tensor(den[:], ps_d2[:],
                                    ceps[:, :1].to_broadcast([P, NH]), OP.add)
            rec = sb.tile([P, NH], F32, tag="rec")
            nc.vector.reciprocal(rec[:], den[:])
            o1 = sb.tile([P, NHF], F32, tag="o1")
            nc.vector.tensor_tensor(
                o1[:].rearrange("p (h f) -> p h f", f=FOUT),
                ps_w2[:].rearrange("p (h f) -> p h f", f=FOUT),
                rec[:, :, None].to_broadcast([P, NH, FOUT]),
                OP.mult)
            xw = sb.tile([P, NHF], F32, tag="xw")
            nc.sync.dma_start(xw[:rows], xloc[w * P:w * P + rows, :])
            nc.vector.tensor_tensor(o1[:rows], o1[:rows], xw[:rows], OP.add)
            nc.vector.tensor_tensor(o1[:rows], o1[:rows], sb_b[:rows], OP.add)
            nmin = sb.tile([P, NHF], F32, tag="nmin")
            nc.vector.tensor_scalar(nmin[:rows], o1[:rows], 0.0, None, OP.min)
            en = sb.tile([P, NHF], F32, tag="en")
            nc.scalar.activation(en[:rows], nmin[:rows], ACT.Exp,
                                 bias=bias_zero[:rows])
            pos = sb.tile([P, NHF], F32, tag="pos")
            nc.vector.tensor_scalar(pos[:rows], o1[:rows], 0.0, None, OP.max)
            nc.vector.tensor_tensor(en[:rows], en[:rows], pos[:rows], OP.add)
            nc.vector.tensor_scalar(en[:rows], en[:rows], -1.0, None, OP.add)
            nc.sync.dma_start(out[w * P:w * P + rows, :], en[:rows])

    nc.compile()
    return nc


def _make_inputs(x, edge_index, w_mat, a_src, a_trg, bias, n_cores):
    n_nodes = x.shape[0]
    npc = n_nodes // n_cores
    t_a, t_b, gidx, rel_arr, selt = _prepare_edges(edge_index, n_nodes, n_cores)
    amat = np.zeros((NHF, 2 * NH), np.float32)
    for h in range(NH):
        amat[h * FOUT:(h + 1) * FOUT, h] = a_src[h]
        amat[h * FOUT:(h + 1) * FOUT, NH + h] = a_trg[h]
    x = np.ascontiguousarray(x, dtype=np.float32)
    in_maps = []
    for c in range(n_cores):
        in_maps.append({
            "x": x,
            "xloc": np.ascontiguousarray(x[c * npc:(c + 1) * npc]),
            "W": np.ascontiguousarray(w_mat, dtype=np.float32),
            "amat": amat,
            "bias": np.ascontiguousarray(bias, dtype=np.float32).reshape(1, NHF),
            "gidx": np.ascontiguousarray(gidx[c]),
            "trg_rel": np.ascontiguousarray(rel_arr[c]),
            "selt": np.ascontiguousarray(selt[c]),
        })
    return t_a, t_b, in_maps


def kernel(x, edge_index, W, a_src, a_trg, bias, _trace=False):
    from concourse.bass_utils import run_bass_kernel_spmd

    n_cores = 8
    x = np.asarray(x)
    n_nodes = x.shape[0]
    t_a, t_b, in_maps = _make_inputs(np.asarray(x), np.asarray(edge_index),
                                     np.asarray(W), np.asarray(a_src),
                                     np.asarray(a_trg), np.asarray(bias),
                                     n_cores)
    nc = build_bass(n_nodes, n_cores, t_a, t_b)
    res = run_bass_kernel_spmd(nc, in_maps, core_ids=list(range(n_cores)),
                               trace=_trace)
    out = np.concatenate([res.results[c]["out"] for c in range(n_cores)], axis=0)
    if _trace:
        kernel.last_results = res
    return out.astype(np.float32)



# revision 9
# speedup vs baseline: 1.1074x; 1.1074x over previous
# GAT (graph attention) layer on 8 Trainium2 NeuronCores — v2.
#
# Target-sharded edges (cores own 6272-aligned target ranges).  Per core:
#   Phase T: gather-table rows [proj(128)|ssrc_hi|strg_hi|ssrc_lo|strg_lo]
#     built from HOST-pretransposed, host-split bf16 hi/lo xT (no on-device
#     transposes or conversions); fp32-accurate scores via 3 hi/lo matmuls.
#     s_trg hi/lo for the core's own targets (stc) from xlocT, kept in SBUF.
#   Phase E: per 128-target window, two ucode dma_gathers fetch [proj|s] rows
#     by src; s_trg per edge via one-hot matmul with a host-streamed
#     transposed one-hot (selt); z -> leaky -> exp(z-24); one matmul per edge
#     tile aggregates [weighted-proj | denom] with targets on partitions
#     (PSUM accumulate, no transposes); per-window results stay in SBUF.
#   Collective: AllReduce(max) of one scalar (global score max M) reproduces
#     the reference's exp(e - e.max()) + 1e-16 epsilon numerics exactly.
#   Phase F: out = elu(W/(D + 1e-16*exp(M-24)) + x + bias), 4 windows/batch.
import sys

import numpy as np

sys.path.insert(0, "/opt/trn_rl_repo")

import ml_dtypes  # noqa: E402

import concourse.bass as bass  # noqa: E402,F401
import concourse.bass_isa as bass_isa  # noqa: E402
import concourse.mybir as mybir  # noqa: E402
import concourse.tile as tile  # noqa: E402
from concourse import bacc  # noqa: E402

P = 128
NH, FOUT = 4, 32
NHF = NH * FOUT  # 128
FIN = 128
ROW = 256  # bf16 elems per table row (512B); 144 used
WEX = NHF + NH  # 132: [weighted proj | ex]
LEAKY = 0.2
SHIFT = 24.0
N_NODES = 50000
N_CORES = 8
NPC = 6272  # 49 * 128, per-core padded target count
NW = 49
NPAD = 50176  # 98 * 512 = 49 * 1024, padded node count
HALF = 25088  # table split (A: [0, 25088), B: [25088, 50176))
TABR = HALF + 1  # +1 pad row (idx 25088) holding -1e4
PADV = -1e4
F32 = mybir.dt.float32
BF16 = mybir.dt.bfloat16
I16 = mybir.dt.int16
AX = mybir.AxisListType
OP = mybir.AluOpType
ACT = mybir.ActivationFunctionType
BF = ml_dtypes.bfloat16


def _wrap16(flat):
    """[..., L] -> dma_gather layout [..., 128, L//16] (16-wrap, replicated)."""
    L = flat.shape[-1]
    w = flat.reshape(flat.shape[:-1] + (L // 16, 16))
    w = np.swapaxes(w, -1, -2)
    return np.tile(w, (1, 1, 8, 1)).reshape(flat.shape[:-1] + (P, L // 16))


def _prepare_edges(edge_index):
    src = np.ascontiguousarray(edge_index[0]).astype(np.int64)
    trg = np.ascontiguousarray(edge_index[1]).astype(np.int64)
    E = src.shape[0]
    wglob = (trg // NPC) * NW + (trg % NPC) // P  # 0..391
    isb = (src >= HALF).astype(np.int64)
    order = np.argsort(wglob * 2 + isb, kind="stable")
    src_s, trg_s, wg_s, isb_s = src[order], trg[order], wglob[order], isb[order]
    nwin = N_CORES * NW
    cnt = np.bincount(wg_s * 2 + isb_s, minlength=2 * nwin)
    t_a = max(1, int(np.ceil(cnt[0::2].max() / P)))
    t_b = max(1, int(np.ceil(cnt[1::2].max() / P)))
    t_eff = t_a + t_b
    gkey = wg_s * 2 + isb_s
    gstart = np.concatenate([[0], np.cumsum(np.bincount(gkey, minlength=2 * nwin))])[:-1]
    jj = np.arange(E) - gstart[gkey]
    t_loc = jj // P
    p_idx = jj % P
    t_idx = np.where(isb_s == 1, t_a + t_loc, t_loc)
    c = wg_s // NW
    wloc = wg_s % NW
    rel = (trg_s % NPC) - wloc * P  # 0..127

    idx_a = np.full((N_CORES, NW, t_a * P), HALF, np.int16)  # pad row
    idx_b = np.full((N_CORES, NW, t_b * P), HALF, np.int16)
    ma = isb_s == 0
    idx_a[c[ma], wloc[ma], t_loc[ma] * P + p_idx[ma]] = src_s[ma].astype(np.int16)
    mb = isb_s == 1
    idx_b[c[mb], wloc[mb], t_loc[mb] * P + p_idx[mb]] = (src_s[mb] - HALF).astype(np.int16)

    rel_arr = np.full((N_CORES, NW * P, t_eff), -1.0, np.float32)
    rel_arr[c, wloc * P + p_idx, t_idx] = rel
    selt = np.zeros((N_CORES, NW * P, t_eff * P), BF)
    selt[c, wloc * P + rel, t_idx * P + p_idx] = 1.0

    ia = _wrap16(idx_a)
    ib = _wrap16(idx_b)
    gidx = np.concatenate([ia, ib], axis=-1).reshape(N_CORES, NW * P, t_eff * 8)
    return t_a, t_b, gidx, rel_arr.astype(BF), selt


def build_bass(t_a, t_b, sim_no_cc=False):
    t_eff = t_a + t_b
    nc = bacc.Bacc("TRN2", target_bir_lowering=False, debug=False,
                   num_devices=N_CORES)

    xTh = nc.dram_tensor("xTh", [P, NPAD], BF16, kind="ExternalInput")
    xTl = nc.dram_tensor("xTl", [P, NPAD], BF16, kind="ExternalInput")
    xlTh = nc.dram_tensor("xlTh", [P, NPC], BF16, kind="ExternalInput")
    xlTl = nc.dram_tensor("xlTl", [P, NPC], BF16, kind="ExternalInput")
    xloc = nc.dram_tensor("xloc", [NPC, FIN], F32, kind="ExternalInput")
    wcatb = nc.dram_tensor("wcatb", [P, 144], BF16, kind="ExternalInput")
    bias_in = nc.dram_tensor("bias", [1, NHF], F32, kind="ExternalInput")
    gidx_in = nc.dram_tensor("gidx", [NW * P, t_eff * 8], I16,
                             kind="ExternalInput")
    rels_in = nc.dram_tensor("rels", [NW * P, t_eff], BF16,
                             kind="ExternalInput")
    selt_in = nc.dram_tensor("selt", [NW * P, t_eff * P], BF16,
                             kind="ExternalInput")
    out = nc.dram_tensor("out", [NPC, NHF], F32, kind="ExternalOutput")

    tab_a = nc.dram_tensor("tab_a", [TABR, ROW], BF16)
    tab_b = nc.dram_tensor("tab_b", [TABR, ROW], BF16)

    with tile.TileContext(nc) as tc:
        with tc.tile_pool(name="const", bufs=1) as const:
            # --- consts and preloads ---
            wc = const.tile([P, 144], BF16)
            nc.sync.dma_start(wc[:], wcatb[:])
            b1 = const.tile([1, NHF], F32)
            nc.sync.dma_start(b1[:], bias_in[:])
            sbias = const.tile([P, NHF], F32)
            nc.gpsimd.partition_broadcast(sbias[:], b1[:])
            c_i32 = const.tile([P, P], mybir.dt.int32)
            nc.gpsimd.iota(c_i32[:], pattern=[[1, P]], base=0,
                           channel_multiplier=0)
            c_bf = const.tile([P, P], BF16)
            nc.vector.tensor_copy(c_bf[:], c_i32[:])
            bias_m24 = const.tile([P, 1], F32)
            nc.gpsimd.memset(bias_m24[:], -SHIFT)
            bias0 = const.tile([P, 1], F32)
            nc.gpsimd.memset(bias0[:], 0.0)
            padrow = const.tile([1, ROW], BF16)
            nc.gpsimd.memset(padrow[:], PADV)
            nc.sync.dma_start(tab_a[HALF:HALF + 1, :], padrow[:])
            nc.sync.dma_start(tab_b[HALF:HALF + 1, :], padrow[:])
            zmax = const.tile([P, t_eff * NH], F32)
            nc.gpsimd.memset(zmax[:], -1e30)
            stc = const.tile([P, NW * 8], BF16)  # [strg_hi(4)|strg_lo(4)]
            stc3 = stc[:].rearrange("p (w c) -> p w c", c=8)
            sWD = const.tile([P, NW * WEX], F32)  # [W(128)|D(4)] per window
            sWD3 = sWD[:].rearrange("p (w c) -> p w c", c=WEX)
            gidx_all = const.tile([P, NW * t_eff * 8], I16)
            nc.sync.dma_start(
                gidx_all[:].rearrange("p (w f) -> p w f", w=NW),
                gidx_in[:].rearrange("(w p) f -> p w f", p=P))
            gidx3 = gidx_all[:].rearrange("p (w f) -> p w f", w=NW)
            rel_all = const.tile([P, NW * t_eff], BF16)
            nc.sync.dma_start(
                rel_all[:].rearrange("p (w f) -> p w f", w=NW),
                rels_in[:].rearrange("(w p) f -> p w f", p=P))
            rel3 = rel_all[:].rearrange("p (w f) -> p w f", w=NW)

            # --- phase T: build gather tables from host-split xT hi/lo ---
            # ps cols 0:128 proj, 128:136 full fp32 scores [ssrc|strg] via
            # hi*W[0:144] + hi*wsa_lo + lo*wsa_hi into the 128:136 window.
            with tc.tile_pool(name="sbT", bufs=3) as sbT, \
                 tc.tile_pool(name="psT", bufs=2, space="PSUM") as psT:
                for sb_i in range(NPAD // 1024):
                    r0 = sb_i * 1024
                    xh = sbT.tile([P, 1024], BF16, tag="xh")
                    nc.sync.dma_start(xh[:], xTh[:, r0:r0 + 1024])
                    xl = sbT.tile([P, 1024], BF16, tag="xl")
                    nc.sync.dma_start(xl[:], xTl[:, r0:r0 + 1024])
                    tabt = sbT.tile([P, 8 * ROW], BF16, tag="tabt")
                    tb3 = tabt[:].rearrange("p (k f) -> p k f", f=ROW)
                    nc.gpsimd.memset(tb3[:, :, 144:256], 0.0)
                    for pair in range(4):
                        ps_p = psT.tile([P, 2 * 128], F32, tag="ps_p")
                        pp3 = ps_p[:].rearrange("p (i f) -> p i f", f=128)
                        ps_s = psT.tile([P, 2 * 8], F32, tag="ps_s")
                        ss3 = ps_s[:].rearrange("p (i f) -> p i f", f=8)
                        for i in range(2):
                            o = (pair * 2 + i) * 128
                            lhi = xh[:, o:o + 128]
                            llo = xl[:, o:o + 128]
                            nc.tensor.matmul(pp3[:, i, :], lhsT=lhi,
                                             rhs=wc[:, 0:128], start=True,
                                             stop=True)
                            nc.tensor.matmul(ss3[:, i, :], lhsT=lhi,
                                             rhs=wc[:, 128:136], start=True,
                                             stop=False)
                            nc.tensor.matmul(ss3[:, i, :], lhsT=lhi,
                                             rhs=wc[:, 136:144], start=False,
                                             stop=False)
                            nc.tensor.matmul(ss3[:, i, :], lhsT=llo,
                                             rhs=wc[:, 128:136], start=False,
                                             stop=True)
                        k = pair * 2
                        nc.scalar.copy(tb3[:, k:k + 2, 0:128], pp3[:])
                        # s slots: [128:136 hi | 136:144 lo], both [ssrc|strg]
                        nc.vector.tensor_copy(tb3[:, k:k + 2, 128:136], ss3)
                        slo = sbT.tile([P, 16], F32, tag="slo")
                        sl3 = slo[:].rearrange("p (i f) -> p i f", f=8)
                        nc.vector.tensor_tensor(sl3, ss3,
                                                tb3[:, k:k + 2, 128:136],
                                                OP.subtract)
                        nc.vector.tensor_copy(tb3[:, k:k + 2, 136:144], sl3)
                    if r0 + 1024 <= HALF:
                        nc.sync.dma_start(
                            tab_a[r0:r0 + 1024, :].rearrange(
                                "(k p) f -> p k f", p=P),
                            tb3[:, :, :])
                    elif r0 >= HALF:
                        nc.sync.dma_start(
                            tab_b[r0 - HALF:r0 - HALF + 1024, :].rearrange(
                                "(k p) f -> p k f", p=P),
                            tb3[:, :, :])
                    else:
                        nc.sync.dma_start(
                            tab_a[r0:HALF, :].rearrange(
                                "(k p) f -> p k f", p=P),
                            tb3[:, 0:4, :])
                        nc.sync.dma_start(
                            tab_b[0:r0 + 1024 - HALF, :].rearrange(
                                "(k p) f -> p k f", p=P),
                            tb3[:, 4:8, :])

                # --- stc: s_trg hi/lo for local targets from xlocT hi/lo ---
                for ci in range(7):
                    c0 = ci * 1024
                    clen = min(1024, NPC - c0)
                    xh = sbT.tile([P, 1024], BF16, tag="xh")
                    nc.sync.dma_start(xh[:, 0:clen], xlTh[:, c0:c0 + clen])
                    xl = sbT.tile([P, 1024], BF16, tag="xl")
                    nc.sync.dma_start(xl[:, 0:clen], xlTl[:, c0:c0 + clen])
                    for pair in range(max(1, clen // 256)):
                        nt2 = min(2, clen // 128 - pair * 2)
                        ps_c = psT.tile([P, 2 * 4], F32, tag="ps_c")
                        cc3 = ps_c[:].rearrange("p (i f) -> p i f", f=4)
                        for i in range(nt2):
                            o = (pair * 2 + i) * 128
                            lhi = xh[:, o:o + 128]
                            llo = xl[:, o:o + 128]
                            nc.tensor.matmul(cc3[:, i, :], lhsT=lhi,
                                             rhs=wc[:, 132:136], start=True,
                                             stop=False)
                            nc.tensor.matmul(cc3[:, i, :], lhsT=lhi,
                                             rhs=wc[:, 140:144], start=False,
                                             stop=False)
                            nc.tensor.matmul(cc3[:, i, :], lhsT=llo,
                                             rhs=wc[:, 132:136], start=False,
                                             stop=True)
                        w0 = c0 // P + pair * 2
                        nc.vector.tensor_copy(stc3[:, w0:w0 + nt2, 0:4],
                                              cc3[:, 0:nt2, :])
                        clo = sbT.tile([P, 8], F32, tag="clo")
                        cl3 = clo[:].rearrange("p (i f) -> p i f", f=4)
                        nc.vector.tensor_tensor(cl3[:, 0:nt2, :],
                                                cc3[:, 0:nt2, :],
                                                stc3[:, w0:w0 + nt2, 0:4],
                                                OP.subtract)
                        nc.vector.tensor_copy(stc3[:, w0:w0 + nt2, 4:8],
                                              cl3[:, 0:nt2, :])

            # --- phase E ---
            with tc.tile_pool(name="sbE", bufs=3) as sbE, \
                 tc.tile_pool(name="sbg", bufs=2) as sbg, \
                 tc.tile_pool(name="psE", bufs=2, space="PSUM") as psE:
                for w in range(NW):
                    gath = sbg.tile([P, t_eff * ROW], BF16, tag="gath")
                    g3 = gath[:].rearrange("p (t c) -> p t c", c=ROW)
                    nc.gpsimd.dma_gather(
                        out_ap=g3[:, 0:t_a, :], in_ap=tab_a[:],
                        idxs_ap=gidx3[:, w, 0:t_a * 8], num_idxs=t_a * P,
                        num_idxs_reg=t_a * P, elem_size=ROW,
                        single_packet=False)
                    nc.gpsimd.dma_gather(
                        out_ap=g3[:, t_a:t_eff, :], in_ap=tab_b[:],
                        idxs_ap=gidx3[:, w, t_a * 8:t_eff * 8],
                        num_idxs=t_b * P, num_idxs_reg=t_b * P, elem_size=ROW,
                        single_packet=False)
                    selt = sbg.tile([P, t_eff * P], BF16, tag="selt")
                    nc.sync.dma_start(selt[:], selt_in[w * P:(w + 1) * P, :])

                    # s_trg per edge via one-hot matmul
                    ps_st = psE.tile([P, t_eff * 8], F32, tag="ps_st")
                    st3 = ps_st[:].rearrange("p (t c) -> p t c", c=8)
                    for t in range(t_eff):
                        nc.tensor.matmul(st3[:, t, :],
                                         lhsT=selt[:, t * P:(t + 1) * P],
                                         rhs=stc3[:, w, :], start=True,
                                         stop=True)

                    # z = (ssrc_hi+strg_hi) + (ssrc_lo+strg_lo)
                    zs8 = sbE.tile([P, t_eff * 8], F32, tag="zs8")
                    z83 = zs8[:].rearrange("p (t h f) -> p t h f", h=2, f=4)
                    gsv = g3[:, :, 128:144].rearrange(
                        "p t (h g f) -> p t h g f", h=2, g=2)
                    nc.vector.tensor_tensor(
                        z83, gsv[:, :, :, 0, :],
                        st3.rearrange("p t (h f) -> p t h f", h=2), OP.add)
                    z = sbE.tile([P, t_eff * NH], F32, tag="z")
                    z3 = z[:].rearrange("p (t c) -> p t c", c=NH)
                    nc.vector.tensor_tensor(z3, z83[:, :, 0, :],
                                            z83[:, :, 1, :], OP.add)
                    nc.vector.tensor_tensor(zmax[:], zmax[:], z[:], OP.max)
                    zl = sbE.tile([P, t_eff * NH], F32, tag="zl")
                    nc.vector.tensor_scalar_mul(zl[:], z[:], LEAKY)
                    nc.vector.tensor_tensor(zl[:], zl[:], z[:], OP.max)

                    wx = sbg.tile([P, t_eff * WEX], BF16, tag="wx")
                    wx3 = wx[:].rearrange("p (t c) -> p t c", c=WEX)
                    nc.scalar.activation(
                        wx3[:, :, 128:132],
                        zl[:].rearrange("p (t c) -> p t c", c=NH),
                        ACT.Exp, bias=bias_m24[:])

                    sel = sbg.tile([P, t_eff * P], BF16, tag="sel")
                    nc.vector.tensor_tensor(
                        sel[:].rearrange("p (t q) -> p t q", q=P),
                        rel3[:, w, :, None].to_broadcast([P, t_eff, P]),
                        c_bf[:, None, :].to_broadcast([P, t_eff, P]),
                        OP.is_equal)
                    nc.vector.tensor_tensor(
                        wx3[:, :, 0:128].rearrange("p t (h f) -> p t h f",
                                                   f=FOUT),
                        g3[:, :, 0:128].rearrange("p t (h f) -> p t h f",
                                                  f=FOUT),
                        wx3[:, :, 128:132][:, :, :, None].to_broadcast(
                            [P, t_eff, NH, FOUT]),
                        OP.mult)

                    ps_o = psE.tile([P, WEX], F32, tag="ps_o")
                    for t in range(t_eff):
                        nc.tensor.matmul(ps_o[:],
                                         lhsT=sel[:, t * P:(t + 1) * P],
                                         rhs=wx3[:, t, :], start=(t == 0),
                                         stop=(t == t_eff - 1))
                    nc.vector.tensor_copy(sWD3[:, w, :], ps_o[:])

                # --- global max + epsilon ---
                zm1 = sbE.tile([P, 1], F32, tag="zm1")
                nc.vector.tensor_reduce(zm1[:], zmax[:], axis=AX.X, op=OP.max)
                zma = sbE.tile([P, 1], F32, tag="zma")
                nc.gpsimd.partition_all_reduce(zma[:], zm1[:], channels=P,
                                               reduce_op=bass_isa.ReduceOp.max)
                with tc.tile_pool(name="dram", bufs=1, space="DRAM") as dram:
                    cc_in = dram.tile([1, 1], F32)
                    cc_out = dram.tile([1, 1], F32)
                    nc.sync.dma_start(cc_in[:], zma[0:1, :])
                    if sim_no_cc:
                        nc.sync.dma_start(cc_out[:], cc_in[:])
                    else:
                        nc.gpsimd.collective_compute(
                            "AllReduce", OP.max,
                            replica_groups=[list(range(N_CORES))],
                            ins=[cc_in.opt()], outs=[cc_out.opt()])
                    zg = sbE.tile([1, 1], F32, tag="zg")
                    nc.sync.dma_start(zg[:], cc_out[:])
                eg = sbE.tile([1, 1], F32, tag="eg")
                nc.vector.tensor_scalar_mul(eg[:], zg[:], LEAKY)
                nc.vector.tensor_tensor(eg[:], eg[:], zg[:], OP.max)
                ce = sbE.tile([1, 1], F32, tag="ce")
                nc.scalar.activation(ce[:], eg[:], ACT.Exp,
                                     bias=bias_m24[:1])
                nc.vector.tensor_scalar_mul(ce[:], ce[:], 1e-16)
                ceps = sbE.tile([P, 1], F32, tag="ceps")
                nc.gpsimd.partition_broadcast(ceps[:], ce[:])

                # --- phase F (single pass over all 49 windows) ---
                sbF = ctxF = tc.tile_pool(name="sbF", bufs=1)
                sbF = sbF.__enter__()
                den = sbF.tile([P, NW * NH], F32, tag="den")
                d3 = den[:].rearrange("p (k c) -> p k c", c=NH)
                nc.vector.tensor_tensor(
                    d3, sWD3[:, :, 128:132],
                    ceps[:, 0:1, None].to_broadcast([P, NW, NH]), OP.add)
                rec = sbF.tile([P, NW * NH], F32, tag="rec")
                nc.vector.reciprocal(rec[:], den[:])
                r3 = rec[:].rearrange("p (k c) -> p k c", c=NH)
                o1 = sbF.tile([P, NW * NHF], F32, tag="o1")
                o14 = o1[:].rearrange("p (k h f) -> p k h f", k=NW, h=NH)
                nc.vector.tensor_tensor(
                    o14,
                    sWD3[:, :, 0:128].rearrange("p k (h f) -> p k h f",
                                                f=FOUT),
                    r3[:, :, :, None].to_broadcast([P, NW, NH, FOUT]),
                    OP.mult)
                xw = sbF.tile([P, NW * NHF], F32, tag="xw")
                x3 = xw[:].rearrange("p (k f) -> p k f", f=NHF)
                nc.sync.dma_start(
                    x3, xloc[:].rearrange("(k p) f -> p k f", p=P))
                nc.vector.tensor_tensor(o1[:], o1[:], xw[:], OP.add)
                o13 = o1[:].rearrange("p (k f) -> p k f", f=NHF)
                nc.vector.tensor_tensor(
                    o13, o13, sbias[:, None, :].to_broadcast([P, NW, NHF]),
                    OP.add)
                ee = sbF.tile([P, NW * NHF], F32, tag="ee")
                nc.scalar.activation(ee[:], o1[:], ACT.Exp, bias=bias0[:])
                nc.vector.tensor_scalar(ee[:], ee[:], -1.0, 0.0,
                                        OP.add, OP.min)
                nc.scalar.activation(xw[:], o1[:], ACT.Relu, bias=bias0[:])
                nc.vector.tensor_tensor(ee[:], ee[:], xw[:], OP.add)
                e3 = ee[:].rearrange("p (k f) -> p k f", f=NHF)
                nc.sync.dma_start(
                    out[:].rearrange("(k p) f -> p k f", p=P), e3)
                ctxF.__exit__(None, None, None)

    nc.compile()
    return nc


def _make_inputs(x, edge_index, w_mat, a_src, a_trg, bias):
    t_a, t_b, gidx, rel_arr, selt = _prepare_edges(edge_index)
    x = np.ascontiguousarray(x, dtype=np.float32)
    xpad = np.zeros((NPAD, FIN), np.float32)
    xpad[:N_NODES] = x
    xT = np.ascontiguousarray(xpad.T)  # [128, 50176] f32
    xTh = xT.astype(BF)
    xTl = (xT - xTh.astype(np.float32)).astype(BF)

    asrc_m = np.zeros((NHF, NH), np.float32)
    atrg_m = np.zeros((NHF, NH), np.float32)
    for h in range(NH):
        asrc_m[h * FOUT:(h + 1) * FOUT, h] = a_src[h]
        atrg_m[h * FOUT:(h + 1) * FOUT, h] = a_trg[h]
    wsa = np.concatenate([w_mat @ asrc_m, w_mat @ atrg_m], axis=1)  # [128, 8]
    wsa_hi = wsa.astype(BF)
    wsa_lo = (wsa - wsa_hi.astype(np.float32)).astype(BF)
    wcatb = np.concatenate(
        [w_mat.astype(BF), wsa_hi, wsa_lo], axis=1)  # [128, 144]

    in_maps = []
    for c in range(N_CORES):
        in_maps.append({
            "xTh": xTh,
            "xTl": xTl,
            "xlTh": np.ascontiguousarray(xTh[:, c * NPC:(c + 1) * NPC]),
            "xlTl": np.ascontiguousarray(xTl[:, c * NPC:(c + 1) * NPC]),
            "xloc": np.ascontiguousarray(xpad[c * NPC:(c + 1) * NPC]),
            "wcatb": wcatb,
            "bias": np.ascontiguousarray(bias, dtype=np.float32).reshape(1, NHF),
            "gidx": np.ascontiguousarray(gidx[c]),
            "rels": np.ascontiguousarray(rel_arr[c]),
            "selt": np.ascontiguousarray(selt[c]),
        })
    return t_a, t_b, in_maps


def kernel(x, edge_index, W, a_src, a_trg, bias, _trace=False):
    from concourse.bass_utils import run_bass_kernel_spmd

    x = np.asarray(x)
    t_a, t_b, in_maps = _make_inputs(x, np.asarray(edge_index),
                                     np.asarray(W, dtype=np.float32),
                                     np.asarray(a_src, dtype=np.float32),
                                     np.asarray(a_trg, dtype=np.float32),
                                     np.asarray(bias, dtype=np.float32))
    nc = build_bass(t_a, t_b)
    res = run_bass_kernel_spmd(nc, in_maps, core_ids=list(range(N_CORES)),
                               trace=_trace)
    parts = []
    for c in range(N_CORES):
        valid = min(NPC, N_NODES - c * NPC)
        parts.append(res.results[c]["out"][:valid])
    out = np.concatenate(parts, axis=0)
    if _trace:
        kernel.last_results = res
    return out.astype(np.float32)


# revision 10
# speedup vs baseline: 1.2252x; 1.1064x over previous
# GAT (graph attention) layer on 8 Trainium2 NeuronCores — v2.
#
# Target-sharded edges (cores own 6272-aligned target ranges).  Per core:
#   Phase T: gather-table rows [proj(128)|ssrc_hi|strg_hi|ssrc_lo|strg_lo]
#     built from HOST-pretransposed, host-split bf16 hi/lo xT (no on-device
#     transposes or conversions); fp32-accurate scores via 3 hi/lo matmuls.
#     s_trg hi/lo for the core's own targets (stc) from xlocT, kept in SBUF.
#   Phase E: per 128-target window, two ucode dma_gathers fetch [proj|s] rows
#     by src; s_trg per edge via one-hot matmul with a host-streamed
#     transposed one-hot (selt); z -> leaky -> exp(z-24); one matmul per edge
#     tile aggregates [weighted-proj | denom] with targets on partitions
#     (PSUM accumulate, no transposes); per-window results stay in SBUF.
#   Collective: AllReduce(max) of one scalar (global score max M) reproduces
#     the reference's exp(e - e.max()) + 1e-16 epsilon numerics exactly.
#   Phase F: out = elu(W/(D + 1e-16*exp(M-24)) + x + bias), 4 windows/batch.
import sys

import numpy as np

sys.path.insert(0, "/opt/trn_rl_repo")

import ml_dtypes  # noqa: E402

import concourse.bass as bass  # noqa: E402,F401
import concourse.bass_isa as bass_isa  # noqa: E402
import concourse.mybir as mybir  # noqa: E402
import concourse.tile as tile  # noqa: E402
from concourse import bacc  # noqa: E402

P = 128
NH, FOUT = 4, 32
NHF = NH * FOUT  # 128
FIN = 128
ROW = 256  # bf16 elems per table row (512B); 144 used
WEX = NHF + NH  # 132: [weighted proj | ex]
LEAKY = 0.2
SHIFT = 24.0
N_NODES = 50000
N_CORES = 8
NPC = 6272  # 49 * 128, per-core padded target count
NW = 49
NPAD = 50176  # 98 * 512 = 49 * 1024, padded node count
HALF = 25088  # table split (A: [0, 25088), B: [25088, 50176))
TABR = HALF + 1  # +1 pad row (idx 25088) holding -1e4
PADV = -1e4
F32 = mybir.dt.float32
BF16 = mybir.dt.bfloat16
I16 = mybir.dt.int16
AX = mybir.AxisListType
OP = mybir.AluOpType
ACT = mybir.ActivationFunctionType
BF = ml_dtypes.bfloat16


def _wrap16(flat):
    """[..., L] -> dma_gather layout [..., 128, L//16] (16-wrap, replicated)."""
    L = flat.shape[-1]
    w = flat.reshape(flat.shape[:-1] + (L // 16, 16))
    w = np.swapaxes(w, -1, -2)
    return np.tile(w, (1, 1, 8, 1)).reshape(flat.shape[:-1] + (P, L // 16))


def _prepare_edges(edge_index):
    src = np.ascontiguousarray(edge_index[0]).astype(np.int64)
    trg = np.ascontiguousarray(edge_index[1]).astype(np.int64)
    E = src.shape[0]
    wglob = (trg // NPC) * NW + (trg % NPC) // P  # 0..391
    isb = (src >= HALF).astype(np.int64)
    order = np.argsort(wglob * 2 + isb, kind="stable")
    src_s, trg_s, wg_s, isb_s = src[order], trg[order], wglob[order], isb[order]
    nwin = N_CORES * NW
    cnt = np.bincount(wg_s * 2 + isb_s, minlength=2 * nwin)
    t_a = max(1, int(np.ceil(cnt[0::2].max() / P)))
    t_b = max(1, int(np.ceil(cnt[1::2].max() / P)))
    t_eff = t_a + t_b
    gkey = wg_s * 2 + isb_s
    gstart = np.concatenate([[0], np.cumsum(np.bincount(gkey, minlength=2 * nwin))])[:-1]
    jj = np.arange(E) - gstart[gkey]
    t_loc = jj // P
    p_idx = jj % P
    t_idx = np.where(isb_s == 1, t_a + t_loc, t_loc)
    c = wg_s // NW
    wloc = wg_s % NW
    rel = (trg_s % NPC) - wloc * P  # 0..127

    idx_a = np.full((N_CORES, NW, t_a * P), HALF, np.int16)  # pad row
    idx_b = np.full((N_CORES, NW, t_b * P), HALF, np.int16)
    ma = isb_s == 0
    idx_a[c[ma], wloc[ma], t_loc[ma] * P + p_idx[ma]] = src_s[ma].astype(np.int16)
    mb = isb_s == 1
    idx_b[c[mb], wloc[mb], t_loc[mb] * P + p_idx[mb]] = (src_s[mb] - HALF).astype(np.int16)

    rel_arr = np.full((N_CORES, NW * P, t_eff), -1.0, np.float32)
    rel_arr[c, wloc * P + p_idx, t_idx] = rel
    selt = np.zeros((N_CORES, NW * P, t_eff * P), BF)
    selt[c, wloc * P + rel, t_idx * P + p_idx] = 1.0

    ia = _wrap16(idx_a)
    ib = _wrap16(idx_b)
    gidx = np.concatenate([ia, ib], axis=-1).reshape(N_CORES, NW * P, t_eff * 8)
    return t_a, t_b, gidx, rel_arr.astype(BF), selt


def build_bass(t_a, t_b, sim_no_cc=False):
    t_eff = t_a + t_b
    nc = bacc.Bacc("TRN2", target_bir_lowering=False, debug=False,
                   num_devices=N_CORES)

    xTh = nc.dram_tensor("xTh", [P, NPAD], BF16, kind="ExternalInput")
    xTl = nc.dram_tensor("xTl", [P, NPAD], BF16, kind="ExternalInput")
    xlTh = nc.dram_tensor("xlTh", [P, NPC], BF16, kind="ExternalInput")
    xlTl = nc.dram_tensor("xlTl", [P, NPC], BF16, kind="ExternalInput")
    xloc = nc.dram_tensor("xloc", [NPC, FIN], F32, kind="ExternalInput")
    wcatb = nc.dram_tensor("wcatb", [P, 144], BF16, kind="ExternalInput")
    bias_in = nc.dram_tensor("bias", [1, NHF], F32, kind="ExternalInput")
    gidx_in = nc.dram_tensor("gidx", [NW * P, t_eff * 8], I16,
                             kind="ExternalInput")
    rels_in = nc.dram_tensor("rels", [NW * P, t_eff], BF16,
                             kind="ExternalInput")
    selt_in = nc.dram_tensor("selt", [NW * P, t_eff * P], BF16,
                             kind="ExternalInput")
    out = nc.dram_tensor("out", [NPC, NHF], F32, kind="ExternalOutput")

    tab_a = nc.dram_tensor("tab_a", [TABR, ROW], BF16)
    tab_b = nc.dram_tensor("tab_b", [TABR, ROW], BF16)

    with tile.TileContext(nc) as tc:
        with tc.tile_pool(name="const", bufs=1) as const:
            # --- consts and preloads ---
            wc = const.tile([P, 144], BF16)
            nc.sync.dma_start(wc[:], wcatb[:])
            b1 = const.tile([1, NHF], F32)
            nc.sync.dma_start(b1[:], bias_in[:])
            sbias = const.tile([P, NHF], F32)
            nc.gpsimd.partition_broadcast(sbias[:], b1[:])
            c_i32 = const.tile([P, P], mybir.dt.int32)
            nc.gpsimd.iota(c_i32[:], pattern=[[1, P]], base=0,
                           channel_multiplier=0)
            c_bf = const.tile([P, P], BF16)
            nc.vector.tensor_copy(c_bf[:], c_i32[:])
            bias_m24 = const.tile([P, 1], F32)
            nc.gpsimd.memset(bias_m24[:], -SHIFT)
            bias0 = const.tile([P, 1], F32)
            nc.gpsimd.memset(bias0[:], 0.0)
            padrow = const.tile([1, ROW], BF16)
            nc.gpsimd.memset(padrow[:], PADV)
            nc.sync.dma_start(tab_a[HALF:HALF + 1, :], padrow[:])
            nc.sync.dma_start(tab_b[HALF:HALF + 1, :], padrow[:])
            zmax = const.tile([P, t_eff * NH], F32)
            nc.gpsimd.memset(zmax[:], -1e30)
            stc = const.tile([P, NW * 8], BF16)  # [strg_hi(4)|strg_lo(4)]
            stc3 = stc[:].rearrange("p (w c) -> p w c", c=8)
            sWD = const.tile([P, NW * WEX], F32)  # [W(128)|D(4)] per window
            sWD3 = sWD[:].rearrange("p (w c) -> p w c", c=WEX)
            gidx_all = const.tile([P, NW * t_eff * 8], I16)
            nc.sync.dma_start(
                gidx_all[:].rearrange("p (w f) -> p w f", w=NW),
                gidx_in[:].rearrange("(w p) f -> p w f", p=P))
            gidx3 = gidx_all[:].rearrange("p (w f) -> p w f", w=NW)
            rel_all = const.tile([P, NW * t_eff], BF16)
            nc.sync.dma_start(
                rel_all[:].rearrange("p (w f) -> p w f", w=NW),
                rels_in[:].rearrange("(w p) f -> p w f", p=P))
            rel3 = rel_all[:].rearrange("p (w f) -> p w f", w=NW)

            # --- phase T: build gather tables from host-split xT hi/lo ---
            # ps cols 0:128 proj, 128:136 full fp32 scores [ssrc|strg] via
            # hi*W[0:144] + hi*wsa_lo + lo*wsa_hi into the 128:136 window.
            with tc.tile_pool(name="sbT", bufs=3) as sbT, \
                 tc.tile_pool(name="psT", bufs=2, space="PSUM") as psT:
                for sb_i in range(NPAD // 1024):
                    r0 = sb_i * 1024
                    xh = sbT.tile([P, 1024], BF16, tag="xh")
                    nc.sync.dma_start(xh[:], xTh[:, r0:r0 + 1024])
                    xl = sbT.tile([P, 1024], BF16, tag="xl")
                    nc.sync.dma_start(xl[:], xTl[:, r0:r0 + 1024])
                    tabt = sbT.tile([P, 8 * ROW], BF16, tag="tabt")
                    tb3 = tabt[:].rearrange("p (k f) -> p k f", f=ROW)
                    nc.gpsimd.memset(tb3[:, :, 144:256], 0.0)
                    for pair in range(4):
                        ps_p = psT.tile([P, 2 * 128], F32, tag="ps_p")
                        pp3 = ps_p[:].rearrange("p (i f) -> p i f", f=128)
                        ps_s = psT.tile([P, 2 * 8], F32, tag="ps_s")
                        ss3 = ps_s[:].rearrange("p (i f) -> p i f", f=8)
                        for i in range(2):
                            o = (pair * 2 + i) * 128
                            lhi = xh[:, o:o + 128]
                            llo = xl[:, o:o + 128]
                            nc.tensor.matmul(pp3[:, i, :], lhsT=lhi,
                                             rhs=wc[:, 0:128], start=True,
                                             stop=True)
                            nc.tensor.matmul(ss3[:, i, :], lhsT=lhi,
                                             rhs=wc[:, 128:136], start=True,
                                             stop=False)
                            nc.tensor.matmul(ss3[:, i, :], lhsT=lhi,
                                             rhs=wc[:, 136:144], start=False,
                                             stop=False)
                            nc.tensor.matmul(ss3[:, i, :], lhsT=llo,
                                             rhs=wc[:, 128:136], start=False,
                                             stop=True)
                        k = pair * 2
                        nc.scalar.copy(tb3[:, k:k + 2, 0:128], pp3[:])
                        # s slots: [128:136 hi | 136:144 lo], both [ssrc|strg]
                        nc.vector.tensor_copy(tb3[:, k:k + 2, 128:136], ss3)
                        slo = sbT.tile([P, 16], F32, tag="slo")
                        sl3 = slo[:].rearrange("p (i f) -> p i f", f=8)
                        nc.vector.tensor_tensor(sl3, ss3,
                                                tb3[:, k:k + 2, 128:136],
                                                OP.subtract)
                        nc.vector.tensor_copy(tb3[:, k:k + 2, 136:144], sl3)
                    if r0 + 1024 <= HALF:
                        nc.sync.dma_start(
                            tab_a[r0:r0 + 1024, :].rearrange(
                                "(k p) f -> p k f", p=P),
                            tb3[:, :, :])
                    elif r0 >= HALF:
                        nc.sync.dma_start(
                            tab_b[r0 - HALF:r0 - HALF + 1024, :].rearrange(
                                "(k p) f -> p k f", p=P),
                            tb3[:, :, :])
                    else:
                        nc.sync.dma_start(
                            tab_a[r0:HALF, :].rearrange(
                                "(k p) f -> p k f", p=P),
                            tb3[:, 0:4, :])
                        nc.sync.dma_start(
                            tab_b[0:r0 + 1024 - HALF, :].rearrange(
                                "(k p) f -> p k f", p=P),
                            tb3[:, 4:8, :])

                # --- stc: s_trg hi/lo for local targets from xlocT hi/lo ---
                for ci in range(7):
                    c0 = ci * 1024
                    clen = min(1024, NPC - c0)
                    xh = sbT.tile([P, 1024], BF16, tag="xh")
                    nc.sync.dma_start(xh[:, 0:clen], xlTh[:, c0:c0 + clen])
                    xl = sbT.tile([P, 1024], BF16, tag="xl")
                    nc.sync.dma_start(xl[:, 0:clen], xlTl[:, c0:c0 + clen])
                    for pair in range(max(1, clen // 256)):
                        nt2 = min(2, clen // 128 - pair * 2)
                        ps_c = psT.tile([P, 2 * 4], F32, tag="ps_c")
                        cc3 = ps_c[:].rearrange("p (i f) -> p i f", f=4)
                        for i in range(nt2):
                            o = (pair * 2 + i) * 128
                            lhi = xh[:, o:o + 128]
                            llo = xl[:, o:o + 128]
                            nc.tensor.matmul(cc3[:, i, :], lhsT=lhi,
                                             rhs=wc[:, 132:136], start=True,
                                             stop=False)
                            nc.tensor.matmul(cc3[:, i, :], lhsT=lhi,
                                             rhs=wc[:, 140:144], start=False,
                                             stop=False)
                            nc.tensor.matmul(cc3[:, i, :], lhsT=llo,
                                             rhs=wc[:, 132:136], start=False,
                                             stop=True)
                        w0 = c0 // P + pair * 2
                        nc.vector.tensor_copy(stc3[:, w0:w0 + nt2, 0:4],
                                              cc3[:, 0:nt2, :])
                        clo = sbT.tile([P, 8], F32, tag="clo")
                        cl3 = clo[:].rearrange("p (i f) -> p i f", f=4)
                        nc.vector.tensor_tensor(cl3[:, 0:nt2, :],
                                                cc3[:, 0:nt2, :],
                                                stc3[:, w0:w0 + nt2, 0:4],
                                                OP.subtract)
                        nc.vector.tensor_copy(stc3[:, w0:w0 + nt2, 4:8],
                                              cl3[:, 0:nt2, :])

            # --- phase E ---
            with tc.tile_pool(name="sbE", bufs=3) as sbE, \
                 tc.tile_pool(name="sbg", bufs=2) as sbg, \
                 tc.tile_pool(name="psE", bufs=2, space="PSUM") as psE:
                for w in range(NW):
                    gath = sbg.tile([P, t_eff * ROW], BF16, tag="gath")
                    g3 = gath[:].rearrange("p (t c) -> p t c", c=ROW)
                    nc.gpsimd.dma_gather(
                        out_ap=g3[:, 0:t_a, :], in_ap=tab_a[:],
                        idxs_ap=gidx3[:, w, 0:t_a * 8], num_idxs=t_a * P,
                        num_idxs_reg=t_a * P, elem_size=ROW,
                        single_packet=False)
                    nc.gpsimd.dma_gather(
                        out_ap=g3[:, t_a:t_eff, :], in_ap=tab_b[:],
                        idxs_ap=gidx3[:, w, t_a * 8:t_eff * 8],
                        num_idxs=t_b * P, num_idxs_reg=t_b * P, elem_size=ROW,
                        single_packet=False)
                    selt = sbg.tile([P, t_eff * P], BF16, tag="selt")
                    nc.sync.dma_start(selt[:], selt_in[w * P:(w + 1) * P, :])

                    # s_trg per edge via one-hot matmul
                    ps_st = psE.tile([P, t_eff * 8], F32, tag="ps_st")
                    st3 = ps_st[:].rearrange("p (t c) -> p t c", c=8)
                    for t in range(t_eff):
                        nc.tensor.matmul(st3[:, t, :],
                                         lhsT=selt[:, t * P:(t + 1) * P],
                                         rhs=stc3[:, w, :], start=True,
                                         stop=True)

                    # z = (ssrc_hi+strg_hi) + (ssrc_lo+strg_lo)
                    zs8 = sbE.tile([P, t_eff * 8], F32, tag="zs8")
                    z83 = zs8[:].rearrange("p (t h f) -> p t h f", h=2, f=4)
                    gsv = g3[:, :, 128:144].rearrange(
                        "p t (h g f) -> p t h g f", h=2, g=2)
                    nc.vector.tensor_tensor(
                        z83, gsv[:, :, :, 0, :],
                        st3.rearrange("p t (h f) -> p t h f", h=2), OP.add)
                    z = sbE.tile([P, t_eff * NH], F32, tag="z")
                    z3 = z[:].rearrange("p (t c) -> p t c", c=NH)
                    nc.vector.tensor_tensor(z3, z83[:, :, 0, :],
                                            z83[:, :, 1, :], OP.add)
                    nc.vector.tensor_tensor(zmax[:], zmax[:], z[:], OP.max)
                    zl = sbE.tile([P, t_eff * NH], F32, tag="zl")
                    nc.vector.tensor_scalar_mul(zl[:], z[:], LEAKY)
                    nc.vector.tensor_tensor(zl[:], zl[:], z[:], OP.max)

                    wx = sbg.tile([P, t_eff * WEX], BF16, tag="wx")
                    wx3 = wx[:].rearrange("p (t c) -> p t c", c=WEX)
                    nc.scalar.activation(
                        wx3[:, :, 128:132],
                        zl[:].rearrange("p (t c) -> p t c", c=NH),
                        ACT.Exp, bias=bias_m24[:])

                    sel = sbg.tile([P, t_eff * P], BF16, tag="sel")
                    nc.vector.tensor_tensor(
                        sel[:].rearrange("p (t q) -> p t q", q=P),
                        rel3[:, w, :, None].to_broadcast([P, t_eff, P]),
                        c_bf[:, None, :].to_broadcast([P, t_eff, P]),
                        OP.is_equal)
                    nc.vector.tensor_tensor(
                        wx3[:, :, 0:128].rearrange("p t (h f) -> p t h f",
                                                   f=FOUT),
                        g3[:, :, 0:128].rearrange("p t (h f) -> p t h f",
                                                  f=FOUT),
                        wx3[:, :, 128:132][:, :, :, None].to_broadcast(
                            [P, t_eff, NH, FOUT]),
                        OP.mult)

                    ps_o = psE.tile([P, WEX], F32, tag="ps_o")
                    for t in range(t_eff):
                        nc.tensor.matmul(ps_o[:],
                                         lhsT=sel[:, t * P:(t + 1) * P],
                                         rhs=wx3[:, t, :], start=(t == 0),
                                         stop=(t == t_eff - 1))
                    nc.scalar.copy(sWD3[:, w, :], ps_o[:])

                # --- global max + epsilon ---
                zm1 = sbE.tile([P, 1], F32, tag="zm1")
                nc.vector.tensor_reduce(zm1[:], zmax[:], axis=AX.X, op=OP.max)
                zma = sbE.tile([P, 1], F32, tag="zma")
                nc.gpsimd.partition_all_reduce(zma[:], zm1[:], channels=P,
                                               reduce_op=bass_isa.ReduceOp.max)
                with tc.tile_pool(name="dram", bufs=1, space="DRAM") as dram:
                    cc_in = dram.tile([1, 1], F32)
                    cc_out = dram.tile([1, 1], F32)
                    nc.sync.dma_start(cc_in[:], zma[0:1, :])
                    if sim_no_cc:
                        nc.sync.dma_start(cc_out[:], cc_in[:])
                    else:
                        nc.gpsimd.collective_compute(
                            "AllReduce", OP.max,
                            replica_groups=[list(range(N_CORES))],
                            ins=[cc_in.opt()], outs=[cc_out.opt()])
                    zg = sbE.tile([1, 1], F32, tag="zg")
                    nc.sync.dma_start(zg[:], cc_out[:])
                eg = sbE.tile([1, 1], F32, tag="eg")
                nc.vector.tensor_scalar_mul(eg[:], zg[:], LEAKY)
                nc.vector.tensor_tensor(eg[:], eg[:], zg[:], OP.max)
                ce = sbE.tile([1, 1], F32, tag="ce")
                nc.scalar.activation(ce[:], eg[:], ACT.Exp,
                                     bias=bias_m24[:1])
                nc.vector.tensor_scalar_mul(ce[:], ce[:], 1e-16)
                ceps = sbE.tile([P, 1], F32, tag="ceps")
                nc.gpsimd.partition_broadcast(ceps[:], ce[:])

                # --- phase F (xpb precomputed; 4 pipelined chunks) ---
                sbF = ctxF = tc.tile_pool(name="sbF", bufs=1)
                sbF = sbF.__enter__()
                xw = sbF.tile([P, NW * NHF], F32, tag="xw")
                x3 = xw[:].rearrange("p (k f) -> p k f", f=NHF)
                nc.sync.dma_start(
                    x3, xloc[:].rearrange("(k p) f -> p k f", p=P))
                # x + bias does not depend on the collective; scheduled early
                nc.vector.tensor_tensor(
                    x3, x3, sbias[:, None, :].to_broadcast([P, NW, NHF]),
                    OP.add)
                den = sbF.tile([P, NW * NH], F32, tag="den")
                d3 = den[:].rearrange("p (k c) -> p k c", c=NH)
                nc.vector.tensor_tensor(
                    d3, sWD3[:, :, 128:132],
                    ceps[:, 0:1, None].to_broadcast([P, NW, NH]), OP.add)
                rec = sbF.tile([P, NW * NH], F32, tag="rec")
                nc.vector.reciprocal(rec[:], den[:])
                r3 = rec[:].rearrange("p (k c) -> p k c", c=NH)
                o1 = sbF.tile([P, NW * NHF], F32, tag="o1")
                o14 = o1[:].rearrange("p (k h f) -> p k h f", k=NW, h=NH)
                ee = sbF.tile([P, NW * NHF], F32, tag="ee")
                pos = sbF.tile([P, NW * NHF], F32, tag="pos")
                bounds = [0, 13, 25, 37, NW]
                for ci in range(4):
                    a, b = bounds[ci], bounds[ci + 1]
                    sl = slice(a * NHF, b * NHF)
                    nc.vector.tensor_tensor(
                        o14[:, a:b],
                        sWD3[:, a:b, 0:128].rearrange("p k (h f) -> p k h f",
                                                      f=FOUT),
                        r3[:, a:b, :, None].to_broadcast(
                            [P, b - a, NH, FOUT]),
                        OP.mult)
                    nc.vector.tensor_tensor(o1[:, sl], o1[:, sl], xw[:, sl],
                                            OP.add)
                    nc.scalar.activation(ee[:, sl], o1[:, sl], ACT.Exp,
                                         bias=bias0[:])
                    nc.scalar.activation(pos[:, sl], o1[:, sl], ACT.Relu,
                                         bias=bias0[:])
                    nc.vector.tensor_scalar(ee[:, sl], ee[:, sl], -1.0, 0.0,
                                            OP.add, OP.min)
                    nc.vector.tensor_tensor(ee[:, sl], ee[:, sl], pos[:, sl],
                                            OP.add)
                    nc.sync.dma_start(
                        out[a * P:b * P, :].rearrange("(k p) f -> p k f",
                                                      p=P),
                        ee[:].rearrange("p (k f) -> p k f", f=NHF)[:, a:b, :])
                ctxF.__exit__(None, None, None)

    nc.compile()
    return nc


def _make_inputs(x, edge_index, w_mat, a_src, a_trg, bias):
    t_a, t_b, gidx, rel_arr, selt = _prepare_edges(edge_index)
    x = np.ascontiguousarray(x, dtype=np.float32)
    xpad = np.zeros((NPAD, FIN), np.float32)
    xpad[:N_NODES] = x
    xT = np.ascontiguousarray(xpad.T)  # [128, 50176] f32
    xTh = xT.astype(BF)
    xTl = (xT - xTh.astype(np.float32)).astype(BF)

    asrc_m = np.zeros((NHF, NH), np.float32)
    atrg_m = np.zeros((NHF, NH), np.float32)
    for h in range(NH):
        asrc_m[h * FOUT:(h + 1) * FOUT, h] = a_src[h]
        atrg_m[h * FOUT:(h + 1) * FOUT, h] = a_trg[h]
    wsa = np.concatenate([w_mat @ asrc_m, w_mat @ atrg_m], axis=1)  # [128, 8]
    wsa_hi = wsa.astype(BF)
    wsa_lo = (wsa - wsa_hi.astype(np.float32)).astype(BF)
    wcatb = np.concatenate(
        [w_mat.astype(BF), wsa_hi, wsa_lo], axis=1)  # [128, 144]

    in_maps = []
    for c in range(N_CORES):
        in_maps.append({
            "xTh": xTh,
            "xTl": xTl,
            "xlTh": np.ascontiguousarray(xTh[:, c * NPC:(c + 1) * NPC]),
            "xlTl": np.ascontiguousarray(xTl[:, c * NPC:(c + 1) * NPC]),
            "xloc": np.ascontiguousarray(xpad[c * NPC:(c + 1) * NPC]),
            "wcatb": wcatb,
            "bias": np.ascontiguousarray(bias, dtype=np.float32).reshape(1, NHF),
            "gidx": np.ascontiguousarray(gidx[c]),
            "rels": np.ascontiguousarray(rel_arr[c]),
            "selt": np.ascontiguousarray(selt[c]),
        })
    return t_a, t_b, in_maps


def kernel(x, edge_index, W, a_src, a_trg, bias, _trace=False):
    from concourse.bass_utils import run_bass_kernel_spmd

    x = np.asarray(x)
    t_a, t_b, in_maps = _make_inputs(x, np.asarray(edge_index),
                                     np.asarray(W, dtype=np.float32),
                                     np.asarray(a_src, dtype=np.float32),
                                     np.asarray(a_trg, dtype=np.float32),
                                     np.asarray(bias, dtype=np.float32))
    nc = build_bass(t_a, t_b)
    res = run_bass_kernel_spmd(nc, in_maps, core_ids=list(range(N_CORES)),
                               trace=_trace)
    parts = []
    for c in range(N_CORES):
        valid = min(NPC, N_NODES - c * NPC)
        parts.append(res.results[c]["out"][:valid])
    out = np.concatenate(parts, axis=0)
    if _trace:
        kernel.last_results = res
    return out.astype(np.float32)


# revision 11
# speedup vs baseline: 1.2263x; 1.0008x over previous
# GAT (graph attention) layer on 8 Trainium2 NeuronCores — v2.
#
# Target-sharded edges (cores own 6272-aligned target ranges).  Per core:
#   Phase T: gather-table rows [proj(128)|ssrc_hi|strg_hi|ssrc_lo|strg_lo]
#     built from HOST-pretransposed, host-split bf16 hi/lo xT (no on-device
#     transposes or conversions); fp32-accurate scores via 3 hi/lo matmuls.
#     s_trg hi/lo for the core's own targets (stc) from xlocT, kept in SBUF.
#   Phase E: per 128-target window, two ucode dma_gathers fetch [proj|s] rows
#     by src; s_trg per edge via one-hot matmul with a host-streamed
#     transposed one-hot (selt); z -> leaky -> exp(z-24); one matmul per edge
#     tile aggregates [weighted-proj | denom] with targets on partitions
#     (PSUM accumulate, no transposes); per-window results stay in SBUF.
#   Collective: AllReduce(max) of one scalar (global score max M) reproduces
#     the reference's exp(e - e.max()) + 1e-16 epsilon numerics exactly.
#   Phase F: out = elu(W/(D + 1e-16*exp(M-24)) + x + bias); x+bias is
#     precomputed during phase E, the rest runs as 4 DVE/Act-pipelined chunks.
import sys

import numpy as np

sys.path.insert(0, "/opt/trn_rl_repo")

import ml_dtypes  # noqa: E402

import concourse.bass as bass  # noqa: E402,F401
import concourse.bass_isa as bass_isa  # noqa: E402
import concourse.mybir as mybir  # noqa: E402
import concourse.tile as tile  # noqa: E402
from concourse import bacc  # noqa: E402

P = 128
NH, FOUT = 4, 32
NHF = NH * FOUT  # 128
FIN = 128
ROW = 256  # bf16 elems per table row (512B); 144 used
WEX = NHF + NH  # 132: [weighted proj | ex]
LEAKY = 0.2
SHIFT = 24.0
N_NODES = 50000
N_CORES = 8
NPC = 6272  # 49 * 128, per-core padded target count
NW = 49
NPAD = 50176  # 98 * 512 = 49 * 1024, padded node count
HALF = 25088  # table split (A: [0, 25088), B: [25088, 50176))
TABR = HALF + 1  # +1 pad row (idx 25088) holding -1e4
PADV = -1e4
F32 = mybir.dt.float32
BF16 = mybir.dt.bfloat16
I16 = mybir.dt.int16
AX = mybir.AxisListType
OP = mybir.AluOpType
ACT = mybir.ActivationFunctionType
BF = ml_dtypes.bfloat16


def _wrap16(flat):
    """[..., L] -> dma_gather layout [..., 128, L//16] (16-wrap, replicated)."""
    L = flat.shape[-1]
    w = flat.reshape(flat.shape[:-1] + (L // 16, 16))
    w = np.swapaxes(w, -1, -2)
    return np.tile(w, (1, 1, 8, 1)).reshape(flat.shape[:-1] + (P, L // 16))


def _prepare_edges(edge_index):
    src = np.ascontiguousarray(edge_index[0]).astype(np.int64)
    trg = np.ascontiguousarray(edge_index[1]).astype(np.int64)
    E = src.shape[0]
    wglob = (trg // NPC) * NW + (trg % NPC) // P  # 0..391
    isb = (src >= HALF).astype(np.int64)
    order = np.argsort(wglob * 2 + isb, kind="stable")
    src_s, trg_s, wg_s, isb_s = src[order], trg[order], wglob[order], isb[order]
    nwin = N_CORES * NW
    cnt = np.bincount(wg_s * 2 + isb_s, minlength=2 * nwin)
    t_a = max(1, int(np.ceil(cnt[0::2].max() / P)))
    t_b = max(1, int(np.ceil(cnt[1::2].max() / P)))
    t_eff = t_a + t_b
    gkey = wg_s * 2 + isb_s
    gstart = np.concatenate([[0], np.cumsum(np.bincount(gkey, minlength=2 * nwin))])[:-1]
    jj = np.arange(E) - gstart[gkey]
    t_loc = jj // P
    p_idx = jj % P
    t_idx = np.where(isb_s == 1, t_a + t_loc, t_loc)
    c = wg_s // NW
    wloc = wg_s % NW
    rel = (trg_s % NPC) - wloc * P  # 0..127

    idx_a = np.full((N_CORES, NW, t_a * P), HALF, np.int16)  # pad row
    idx_b = np.full((N_CORES, NW, t_b * P), HALF, np.int16)
    ma = isb_s == 0
    idx_a[c[ma], wloc[ma], t_loc[ma] * P + p_idx[ma]] = src_s[ma].astype(np.int16)
    mb = isb_s == 1
    idx_b[c[mb], wloc[mb], t_loc[mb] * P + p_idx[mb]] = (src_s[mb] - HALF).astype(np.int16)

    rel_arr = np.full((N_CORES, NW * P, t_eff), -1.0, np.float32)
    rel_arr[c, wloc * P + p_idx, t_idx] = rel
    selt = np.zeros((N_CORES, NW * P, t_eff * P), BF)
    selt[c, wloc * P + rel, t_idx * P + p_idx] = 1.0

    ia = _wrap16(idx_a)
    ib = _wrap16(idx_b)
    gidx = np.concatenate([ia, ib], axis=-1).reshape(N_CORES, NW * P, t_eff * 8)
    return t_a, t_b, gidx, rel_arr.astype(BF), selt


def build_bass(t_a, t_b, sim_no_cc=False):
    t_eff = t_a + t_b
    nc = bacc.Bacc("TRN2", target_bir_lowering=False, debug=False,
                   num_devices=N_CORES)

    xTh = nc.dram_tensor("xTh", [P, NPAD], BF16, kind="ExternalInput")
    xTl = nc.dram_tensor("xTl", [P, NPAD], BF16, kind="ExternalInput")
    xlTh = nc.dram_tensor("xlTh", [P, NPC], BF16, kind="ExternalInput")
    xlTl = nc.dram_tensor("xlTl", [P, NPC], BF16, kind="ExternalInput")
    xloc = nc.dram_tensor("xloc", [NPC, FIN], F32, kind="ExternalInput")
    wcatb = nc.dram_tensor("wcatb", [P, 144], BF16, kind="ExternalInput")
    bias_in = nc.dram_tensor("bias", [1, NHF], F32, kind="ExternalInput")
    gidx_in = nc.dram_tensor("gidx", [NW * P, t_eff * 8], I16,
                             kind="ExternalInput")
    rels_in = nc.dram_tensor("rels", [NW * P, t_eff], BF16,
                             kind="ExternalInput")
    selt_in = nc.dram_tensor("selt", [NW * P, t_eff * P], BF16,
                             kind="ExternalInput")
    out = nc.dram_tensor("out", [NPC, NHF], F32, kind="ExternalOutput")

    tab_a = nc.dram_tensor("tab_a", [TABR, ROW], BF16)
    tab_b = nc.dram_tensor("tab_b", [TABR, ROW], BF16)

    with tile.TileContext(nc) as tc:
        with tc.tile_pool(name="const", bufs=1) as const:
            # --- consts and preloads ---
            wc = const.tile([P, 144], BF16)
            nc.sync.dma_start(wc[:], wcatb[:])
            b1 = const.tile([1, NHF], F32)
            nc.sync.dma_start(b1[:], bias_in[:])
            sbias = const.tile([P, NHF], F32)
            nc.gpsimd.partition_broadcast(sbias[:], b1[:])
            c_i32 = const.tile([P, P], mybir.dt.int32)
            nc.gpsimd.iota(c_i32[:], pattern=[[1, P]], base=0,
                           channel_multiplier=0)
            c_bf = const.tile([P, P], BF16)
            nc.vector.tensor_copy(c_bf[:], c_i32[:])
            bias_m24 = const.tile([P, 1], F32)
            nc.gpsimd.memset(bias_m24[:], -SHIFT)
            bias0 = const.tile([P, 1], F32)
            nc.gpsimd.memset(bias0[:], 0.0)
            padrow = const.tile([1, ROW], BF16)
            nc.gpsimd.memset(padrow[:], PADV)
            nc.sync.dma_start(tab_a[HALF:HALF + 1, :], padrow[:])
            nc.sync.dma_start(tab_b[HALF:HALF + 1, :], padrow[:])
            zmax = const.tile([P, t_eff * NH], F32)
            nc.gpsimd.memset(zmax[:], -1e30)
            stc = const.tile([P, NW * 8], BF16)  # [strg_hi(4)|strg_lo(4)]
            stc3 = stc[:].rearrange("p (w c) -> p w c", c=8)
            sWD = const.tile([P, NW * WEX], F32)  # [W(128)|D(4)] per window
            sWD3 = sWD[:].rearrange("p (w c) -> p w c", c=WEX)
            gidx_all = const.tile([P, NW * t_eff * 8], I16)
            nc.sync.dma_start(
                gidx_all[:].rearrange("p (w f) -> p w f", w=NW),
                gidx_in[:].rearrange("(w p) f -> p w f", p=P))
            gidx3 = gidx_all[:].rearrange("p (w f) -> p w f", w=NW)
            rel_all = const.tile([P, NW * t_eff], BF16)
            nc.sync.dma_start(
                rel_all[:].rearrange("p (w f) -> p w f", w=NW),
                rels_in[:].rearrange("(w p) f -> p w f", p=P))
            rel3 = rel_all[:].rearrange("p (w f) -> p w f", w=NW)

            # --- phase T: build gather tables from host-split xT hi/lo ---
            # ps cols 0:128 proj, 128:136 full fp32 scores [ssrc|strg] via
            # hi*W[0:144] + hi*wsa_lo + lo*wsa_hi into the 128:136 window.
            with tc.tile_pool(name="sbT", bufs=3) as sbT, \
                 tc.tile_pool(name="psT", bufs=2, space="PSUM") as psT:
                for sb_i in range(NPAD // 1024):
                    r0 = sb_i * 1024
                    xh = sbT.tile([P, 1024], BF16, tag="xh")
                    nc.sync.dma_start(xh[:], xTh[:, r0:r0 + 1024])
                    xl = sbT.tile([P, 1024], BF16, tag="xl")
                    nc.sync.dma_start(xl[:], xTl[:, r0:r0 + 1024])
                    tabt = sbT.tile([P, 8 * ROW], BF16, tag="tabt")
                    tb3 = tabt[:].rearrange("p (k f) -> p k f", f=ROW)
                    nc.gpsimd.memset(tb3[:, :, 144:256], 0.0)
                    for pair in range(4):
                        ps_p = psT.tile([P, 2 * 128], F32, tag="ps_p")
                        pp3 = ps_p[:].rearrange("p (i f) -> p i f", f=128)
                        ps_s = psT.tile([P, 2 * 8], F32, tag="ps_s")
                        ss3 = ps_s[:].rearrange("p (i f) -> p i f", f=8)
                        for i in range(2):
                            o = (pair * 2 + i) * 128
                            lhi = xh[:, o:o + 128]
                            llo = xl[:, o:o + 128]
                            nc.tensor.matmul(pp3[:, i, :], lhsT=lhi,
                                             rhs=wc[:, 0:128], start=True,
                                             stop=True)
                            nc.tensor.matmul(ss3[:, i, :], lhsT=lhi,
                                             rhs=wc[:, 128:136], start=True,
                                             stop=False)
                            nc.tensor.matmul(ss3[:, i, :], lhsT=lhi,
                                             rhs=wc[:, 136:144], start=False,
                                             stop=False)
                            nc.tensor.matmul(ss3[:, i, :], lhsT=llo,
                                             rhs=wc[:, 128:136], start=False,
                                             stop=True)
                        k = pair * 2
                        nc.scalar.copy(tb3[:, k:k + 2, 0:128], pp3[:])
                        # s slots: [128:136 hi | 136:144 lo], both [ssrc|strg]
                        nc.vector.tensor_copy(tb3[:, k:k + 2, 128:136], ss3)
                        slo = sbT.tile([P, 16], F32, tag="slo")
                        sl3 = slo[:].rearrange("p (i f) -> p i f", f=8)
                        nc.vector.tensor_tensor(sl3, ss3,
                                                tb3[:, k:k + 2, 128:136],
                                                OP.subtract)
                        nc.vector.tensor_copy(tb3[:, k:k + 2, 136:144], sl3)
                    if r0 + 1024 <= HALF:
                        nc.sync.dma_start(
                            tab_a[r0:r0 + 1024, :].rearrange(
                                "(k p) f -> p k f", p=P),
                            tb3[:, :, :])
                    elif r0 >= HALF:
                        nc.sync.dma_start(
                            tab_b[r0 - HALF:r0 - HALF + 1024, :].rearrange(
                                "(k p) f -> p k f", p=P),
                            tb3[:, :, :])
                    else:
                        nc.sync.dma_start(
                            tab_a[r0:HALF, :].rearrange(
                                "(k p) f -> p k f", p=P),
                            tb3[:, 0:4, :])
                        nc.sync.dma_start(
                            tab_b[0:r0 + 1024 - HALF, :].rearrange(
                                "(k p) f -> p k f", p=P),
                            tb3[:, 4:8, :])

                # --- stc: s_trg hi/lo for local targets from xlocT hi/lo ---
                for ci in range(7):
                    c0 = ci * 1024
                    clen = min(1024, NPC - c0)
                    xh = sbT.tile([P, 1024], BF16, tag="xh")
                    nc.sync.dma_start(xh[:, 0:clen], xlTh[:, c0:c0 + clen])
                    xl = sbT.tile([P, 1024], BF16, tag="xl")
                    nc.sync.dma_start(xl[:, 0:clen], xlTl[:, c0:c0 + clen])
                    for pair in range(max(1, clen // 256)):
                        nt2 = min(2, clen // 128 - pair * 2)
                        ps_c = psT.tile([P, 2 * 4], F32, tag="ps_c")
                        cc3 = ps_c[:].rearrange("p (i f) -> p i f", f=4)
                        for i in range(nt2):
                            o = (pair * 2 + i) * 128
                            lhi = xh[:, o:o + 128]
                            llo = xl[:, o:o + 128]
                            nc.tensor.matmul(cc3[:, i, :], lhsT=lhi,
                                             rhs=wc[:, 132:136], start=True,
                                             stop=False)
                            nc.tensor.matmul(cc3[:, i, :], lhsT=lhi,
                                             rhs=wc[:, 140:144], start=False,
                                             stop=False)
                            nc.tensor.matmul(cc3[:, i, :], lhsT=llo,
                                             rhs=wc[:, 132:136], start=False,
                                             stop=True)
                        w0 = c0 // P + pair * 2
                        nc.vector.tensor_copy(stc3[:, w0:w0 + nt2, 0:4],
                                              cc3[:, 0:nt2, :])
                        clo = sbT.tile([P, 8], F32, tag="clo")
                        cl3 = clo[:].rearrange("p (i f) -> p i f", f=4)
                        nc.vector.tensor_tensor(cl3[:, 0:nt2, :],
                                                cc3[:, 0:nt2, :],
                                                stc3[:, w0:w0 + nt2, 0:4],
                                                OP.subtract)
                        nc.vector.tensor_copy(stc3[:, w0:w0 + nt2, 4:8],
                                              cl3[:, 0:nt2, :])

            # --- phase E ---
            with tc.tile_pool(name="sbE", bufs=3) as sbE, \
                 tc.tile_pool(name="sbg", bufs=2) as sbg, \
                 tc.tile_pool(name="psE", bufs=2, space="PSUM") as psE:
                for w in range(NW):
                    gath = sbg.tile([P, t_eff * ROW], BF16, tag="gath")
                    g3 = gath[:].rearrange("p (t c) -> p t c", c=ROW)
                    nc.gpsimd.dma_gather(
                        out_ap=g3[:, 0:t_a, :], in_ap=tab_a[:],
                        idxs_ap=gidx3[:, w, 0:t_a * 8], num_idxs=t_a * P,
                        num_idxs_reg=t_a * P, elem_size=ROW,
                        single_packet=False)
                    nc.gpsimd.dma_gather(
                        out_ap=g3[:, t_a:t_eff, :], in_ap=tab_b[:],
                        idxs_ap=gidx3[:, w, t_a * 8:t_eff * 8],
                        num_idxs=t_b * P, num_idxs_reg=t_b * P, elem_size=ROW,
                        single_packet=False)
                    selt = sbg.tile([P, t_eff * P], BF16, tag="selt")
                    nc.sync.dma_start(selt[:], selt_in[w * P:(w + 1) * P, :])

                    # s_trg per edge via one-hot matmul
                    ps_st = psE.tile([P, t_eff * 8], F32, tag="ps_st")
                    st3 = ps_st[:].rearrange("p (t c) -> p t c", c=8)
                    for t in range(t_eff):
                        nc.tensor.matmul(st3[:, t, :],
                                         lhsT=selt[:, t * P:(t + 1) * P],
                                         rhs=stc3[:, w, :], start=True,
                                         stop=True)

                    # z = (ssrc_hi+strg_hi) + (ssrc_lo+strg_lo)
                    zs8 = sbE.tile([P, t_eff * 8], F32, tag="zs8")
                    z83 = zs8[:].rearrange("p (t h f) -> p t h f", h=2, f=4)
                    gsv = g3[:, :, 128:144].rearrange(
                        "p t (h g f) -> p t h g f", h=2, g=2)
                    nc.vector.tensor_tensor(
                        z83, gsv[:, :, :, 0, :],
                        st3.rearrange("p t (h f) -> p t h f", h=2), OP.add)
                    z = sbE.tile([P, t_eff * NH], F32, tag="z")
                    z3 = z[:].rearrange("p (t c) -> p t c", c=NH)
                    nc.vector.tensor_tensor(z3, z83[:, :, 0, :],
                                            z83[:, :, 1, :], OP.add)
                    nc.vector.tensor_tensor(zmax[:], zmax[:], z[:], OP.max)
                    zl = sbE.tile([P, t_eff * NH], F32, tag="zl")
                    nc.vector.tensor_scalar_mul(zl[:], z[:], LEAKY)
                    nc.vector.tensor_tensor(zl[:], zl[:], z[:], OP.max)

                    wx = sbg.tile([P, t_eff * WEX], BF16, tag="wx")
                    wx3 = wx[:].rearrange("p (t c) -> p t c", c=WEX)
                    nc.scalar.activation(
                        wx3[:, :, 128:132],
                        zl[:].rearrange("p (t c) -> p t c", c=NH),
                        ACT.Exp, bias=bias_m24[:])

                    sel = sbg.tile([P, t_eff * P], BF16, tag="sel")
                    nc.vector.tensor_tensor(
                        sel[:].rearrange("p (t q) -> p t q", q=P),
                        rel3[:, w, :, None].to_broadcast([P, t_eff, P]),
                        c_bf[:, None, :].to_broadcast([P, t_eff, P]),
                        OP.is_equal)
                    nc.vector.tensor_tensor(
                        wx3[:, :, 0:128].rearrange("p t (h f) -> p t h f",
                                                   f=FOUT),
                        g3[:, :, 0:128].rearrange("p t (h f) -> p t h f",
                                                  f=FOUT),
                        wx3[:, :, 128:132][:, :, :, None].to_broadcast(
                            [P, t_eff, NH, FOUT]),
                        OP.mult)

                    ps_o = psE.tile([P, WEX], F32, tag="ps_o")
                    for t in range(t_eff):
                        nc.tensor.matmul(ps_o[:],
                                         lhsT=sel[:, t * P:(t + 1) * P],
                                         rhs=wx3[:, t, :], start=(t == 0),
                                         stop=(t == t_eff - 1))
                    nc.scalar.copy(sWD3[:, w, :], ps_o[:])

                # --- global max + epsilon ---
                zm1 = sbE.tile([P, 1], F32, tag="zm1")
                nc.vector.tensor_reduce(zm1[:], zmax[:], axis=AX.X, op=OP.max)
                zma = sbE.tile([P, 1], F32, tag="zma")
                nc.gpsimd.partition_all_reduce(zma[:], zm1[:], channels=P,
                                               reduce_op=bass_isa.ReduceOp.max)
                with tc.tile_pool(name="dram", bufs=1, space="DRAM") as dram:
                    cc_in = dram.tile([1, 1], F32)
                    cc_out = dram.tile([1, 1], F32)
                    nc.sync.dma_start(cc_in[:], zma[0:1, :])
                    if sim_no_cc:
                        nc.sync.dma_start(cc_out[:], cc_in[:])
                    else:
                        nc.gpsimd.collective_compute(
                            "AllReduce", OP.max,
                            replica_groups=[list(range(N_CORES))],
                            ins=[cc_in.opt()], outs=[cc_out.opt()])
                    zg = sbE.tile([1, 1], F32, tag="zg")
                    nc.sync.dma_start(zg[:], cc_out[:])
                eg = sbE.tile([1, 1], F32, tag="eg")
                nc.vector.tensor_scalar_mul(eg[:], zg[:], LEAKY)
                nc.vector.tensor_tensor(eg[:], eg[:], zg[:], OP.max)
                ce = sbE.tile([1, 1], F32, tag="ce")
                nc.scalar.activation(ce[:], eg[:], ACT.Exp,
                                     bias=bias_m24[:1])
                nc.vector.tensor_scalar_mul(ce[:], ce[:], 1e-16)
                ceps = sbE.tile([P, 1], F32, tag="ceps")
                nc.gpsimd.partition_broadcast(ceps[:], ce[:])

                # --- phase F (xpb precomputed; 4 pipelined chunks) ---
                sbF = ctxF = tc.tile_pool(name="sbF", bufs=1)
                sbF = sbF.__enter__()
                xw = sbF.tile([P, NW * NHF], F32, tag="xw")
                x3 = xw[:].rearrange("p (k f) -> p k f", f=NHF)
                nc.sync.dma_start(
                    x3, xloc[:].rearrange("(k p) f -> p k f", p=P))
                # x + bias does not depend on the collective; scheduled early
                nc.vector.tensor_tensor(
                    x3, x3, sbias[:, None, :].to_broadcast([P, NW, NHF]),
                    OP.add)
                den = sbF.tile([P, NW * NH], F32, tag="den")
                d3 = den[:].rearrange("p (k c) -> p k c", c=NH)
                nc.vector.tensor_tensor(
                    d3, sWD3[:, :, 128:132],
                    ceps[:, 0:1, None].to_broadcast([P, NW, NH]), OP.add)
                rec = sbF.tile([P, NW * NH], F32, tag="rec")
                nc.vector.reciprocal(rec[:], den[:])
                r3 = rec[:].rearrange("p (k c) -> p k c", c=NH)
                o1 = sbF.tile([P, NW * NHF], F32, tag="o1")
                o14 = o1[:].rearrange("p (k h f) -> p k h f", k=NW, h=NH)
                ee = sbF.tile([P, NW * NHF], F32, tag="ee")
                pos = sbF.tile([P, NW * NHF], F32, tag="pos")
                bounds = [0, 13, 25, 37, NW]
                for ci in range(4):
                    a, b = bounds[ci], bounds[ci + 1]
                    sl = slice(a * NHF, b * NHF)
                    nc.vector.tensor_tensor(
                        o14[:, a:b],
                        sWD3[:, a:b, 0:128].rearrange("p k (h f) -> p k h f",
                                                      f=FOUT),
                        r3[:, a:b, :, None].to_broadcast(
                            [P, b - a, NH, FOUT]),
                        OP.mult)
                    nc.vector.tensor_tensor(o1[:, sl], o1[:, sl], xw[:, sl],
                                            OP.add)
                    nc.scalar.activation(ee[:, sl], o1[:, sl], ACT.Exp,
                                         bias=bias0[:])
                    nc.scalar.activation(pos[:, sl], o1[:, sl], ACT.Relu,
                                         bias=bias0[:])
                    nc.vector.tensor_scalar(ee[:, sl], ee[:, sl], -1.0, 0.0,
                                            OP.add, OP.min)
                    nc.vector.tensor_tensor(ee[:, sl], ee[:, sl], pos[:, sl],
                                            OP.add)
                    nc.sync.dma_start(
                        out[a * P:b * P, :].rearrange("(k p) f -> p k f",
                                                      p=P),
                        ee[:].rearrange("p (k f) -> p k f", f=NHF)[:, a:b, :])
                ctxF.__exit__(None, None, None)

    nc.compile()
    return nc


def _make_inputs(x, edge_index, w_mat, a_src, a_trg, bias):
    t_a, t_b, gidx, rel_arr, selt = _prepare_edges(edge_index)
    x = np.ascontiguousarray(x, dtype=np.float32)
    xpad = np.zeros((NPAD, FIN), np.float32)
    xpad[:N_NODES] = x
    xT = np.ascontiguousarray(xpad.T)  # [128, 50176] f32
    xTh = xT.astype(BF)
    xTl = (xT - xTh.astype(np.float32)).astype(BF)

    asrc_m = np.zeros((NHF, NH), np.float32)
    atrg_m = np.zeros((NHF, NH), np.float32)
    for h in range(NH):
        asrc_m[h * FOUT:(h + 1) * FOUT, h] = a_src[h]
        atrg_m[h * FOUT:(h + 1) * FOUT, h] = a_trg[h]
    wsa = np.concatenate([w_mat @ asrc_m, w_mat @ atrg_m], axis=1)  # [128, 8]
    wsa_hi = wsa.astype(BF)
    wsa_lo = (wsa - wsa_hi.astype(np.float32)).astype(BF)
    wcatb = np.concatenate(
        [w_mat.astype(BF), wsa_hi, wsa_lo], axis=1)  # [128, 144]

    in_maps = []
    for c in range(N_CORES):
        in_maps.append({
            "xTh": xTh,
            "xTl": xTl,
            "xlTh": np.ascontiguousarray(xTh[:, c * NPC:(c + 1) * NPC]),
            "xlTl": np.ascontiguousarray(xTl[:, c * NPC:(c + 1) * NPC]),
            "xloc": np.ascontiguousarray(xpad[c * NPC:(c + 1) * NPC]),
            "wcatb": wcatb,
            "bias": np.ascontiguousarray(bias, dtype=np.float32).reshape(1, NHF),
            "gidx": np.ascontiguousarray(gidx[c]),
            "rels": np.ascontiguousarray(rel_arr[c]),
            "selt": np.ascontiguousarray(selt[c]),
        })
    return t_a, t_b, in_maps


def kernel(x, edge_index, W, a_src, a_trg, bias, _trace=False):
    from concourse.bass_utils import run_bass_kernel_spmd

    x = np.asarray(x)
    t_a, t_b, in_maps = _make_inputs(x, np.asarray(edge_index),
                                     np.asarray(W, dtype=np.float32),
                                     np.asarray(a_src, dtype=np.float32),
                                     np.asarray(a_trg, dtype=np.float32),
                                     np.asarray(bias, dtype=np.float32))
    nc = build_bass(t_a, t_b)
    res = run_bass_kernel_spmd(nc, in_maps, core_ids=list(range(N_CORES)),
                               trace=_trace)
    parts = []
    for c in range(N_CORES):
        valid = min(NPC, N_NODES - c * NPC)
        parts.append(res.results[c]["out"][:valid])
    out = np.concatenate(parts, axis=0)
    if _trace:
        kernel.last_results = res
    return out.astype(np.float32)


# revision 14
# speedup vs baseline: 1.2534x; 1.0221x over previous
# GAT (graph attention) layer on 8 Trainium2 NeuronCores — v2.
#
# Target-sharded edges (cores own 6272-aligned target ranges).  Per core:
#   Phase T: gather-table rows [proj(128)|ssrc_hi|strg_hi|ssrc_lo|strg_lo]
#     built from HOST-pretransposed, host-split bf16 hi/lo xT (no on-device
#     transposes or conversions); fp32-accurate scores via 3 hi/lo matmuls.
#     s_trg hi/lo for the core's own targets (stc) from xlocT, kept in SBUF.
#   Phase E: per 128-target window, two ucode dma_gathers fetch [proj|s] rows
#     by src; s_trg per edge via one-hot matmul with a host-streamed
#     transposed one-hot (selt); z -> leaky -> exp(z-24); one matmul per edge
#     tile aggregates [weighted-proj | denom] with targets on partitions
#     (PSUM accumulate, no transposes); per-window results stay in SBUF.
#   Collective: AllReduce(max) of one scalar (global score max M) reproduces
#     the reference's exp(e - e.max()) + 1e-16 epsilon numerics exactly.
#   Phase F: out = elu(W/(D + 1e-16*exp(M-24)) + x + bias); x+bias is
#     precomputed during phase E, the rest runs as 4 DVE/Act-pipelined chunks.
import sys

import numpy as np

sys.path.insert(0, "/opt/trn_rl_repo")

import ml_dtypes  # noqa: E402

import concourse.bass as bass  # noqa: E402,F401
import concourse.bass_isa as bass_isa  # noqa: E402
import concourse.mybir as mybir  # noqa: E402
import concourse.tile as tile  # noqa: E402
from concourse import bacc  # noqa: E402

P = 128
NH, FOUT = 4, 32
NHF = NH * FOUT  # 128
FIN = 128
ROW = 256  # bf16 elems per table row (512B); 144 used
WEX = NHF + NH  # 132: [weighted proj | ex]
LEAKY = 0.2
SHIFT = 24.0
N_NODES = 50000
N_CORES = 8
NPC = 6272  # 49 * 128, per-core padded target count
NW = 49
NPAD = 50176  # 98 * 512 = 49 * 1024, padded node count
HALF = 25088  # table split (A: [0, 25088), B: [25088, 50176))
TABR = HALF + 1  # +1 pad row (idx 25088) holding -1e4
PADV = -1e4
F32 = mybir.dt.float32
BF16 = mybir.dt.bfloat16
I16 = mybir.dt.int16
AX = mybir.AxisListType
OP = mybir.AluOpType
ACT = mybir.ActivationFunctionType
BF = ml_dtypes.bfloat16


def _wrap16(flat):
    """[..., L] -> dma_gather layout [..., 128, L//16] (16-wrap, replicated)."""
    L = flat.shape[-1]
    w = flat.reshape(flat.shape[:-1] + (L // 16, 16))
    w = np.swapaxes(w, -1, -2)
    return np.tile(w, (1, 1, 8, 1)).reshape(flat.shape[:-1] + (P, L // 16))


def _prepare_edges(edge_index):
    src = np.ascontiguousarray(edge_index[0]).astype(np.int64)
    trg = np.ascontiguousarray(edge_index[1]).astype(np.int64)
    E = src.shape[0]
    wglob = (trg // NPC) * NW + (trg % NPC) // P  # 0..391
    isb = (src >= HALF).astype(np.int64)
    order = np.argsort(wglob * 2 + isb, kind="stable")
    src_s, trg_s, wg_s, isb_s = src[order], trg[order], wglob[order], isb[order]
    nwin = N_CORES * NW
    cnt = np.bincount(wg_s * 2 + isb_s, minlength=2 * nwin)
    t_a = max(1, int(np.ceil(cnt[0::2].max() / P)))
    t_b = max(1, int(np.ceil(cnt[1::2].max() / P)))
    t_eff = t_a + t_b
    gkey = wg_s * 2 + isb_s
    gstart = np.concatenate([[0], np.cumsum(np.bincount(gkey, minlength=2 * nwin))])[:-1]
    jj = np.arange(E) - gstart[gkey]
    t_loc = jj // P
    p_idx = jj % P
    t_idx = np.where(isb_s == 1, t_a + t_loc, t_loc)
    c = wg_s // NW
    wloc = wg_s % NW
    rel = (trg_s % NPC) - wloc * P  # 0..127

    idx_a = np.full((N_CORES, NW, t_a * P), HALF, np.int16)  # pad row
    idx_b = np.full((N_CORES, NW, t_b * P), HALF, np.int16)
    ma = isb_s == 0
    idx_a[c[ma], wloc[ma], t_loc[ma] * P + p_idx[ma]] = src_s[ma].astype(np.int16)
    mb = isb_s == 1
    idx_b[c[mb], wloc[mb], t_loc[mb] * P + p_idx[mb]] = (src_s[mb] - HALF).astype(np.int16)

    rel_arr = np.full((N_CORES, NW * P, t_eff), -1.0, np.float32)
    rel_arr[c, wloc * P + p_idx, t_idx] = rel
    selt = np.zeros((N_CORES, NW * P, t_eff * P), BF)
    selt[c, wloc * P + rel, t_idx * P + p_idx] = 1.0

    ia = _wrap16(idx_a)
    ib = _wrap16(idx_b)
    gidx = np.concatenate([ia, ib], axis=-1).reshape(N_CORES, NW * P, t_eff * 8)
    return t_a, t_b, gidx, rel_arr.astype(BF), selt


def build_bass(t_a, t_b, sim_no_cc=False):
    t_eff = t_a + t_b
    nc = bacc.Bacc("TRN2", target_bir_lowering=False, debug=False,
                   num_devices=N_CORES)

    xTh = nc.dram_tensor("xTh", [P, NPAD], BF16, kind="ExternalInput")
    xTl = nc.dram_tensor("xTl", [P, NPAD], BF16, kind="ExternalInput")
    xlTh = nc.dram_tensor("xlTh", [P, NPC], BF16, kind="ExternalInput")
    xlTl = nc.dram_tensor("xlTl", [P, NPC], BF16, kind="ExternalInput")
    xloc = nc.dram_tensor("xloc", [NPC, FIN], F32, kind="ExternalInput")
    wcatb = nc.dram_tensor("wcatb", [P, 144], BF16, kind="ExternalInput")
    bias_in = nc.dram_tensor("bias", [1, NHF], F32, kind="ExternalInput")
    gidx_in = nc.dram_tensor("gidx", [NW * P, t_eff * 8], I16,
                             kind="ExternalInput")
    rels_in = nc.dram_tensor("rels", [NW * P, t_eff], BF16,
                             kind="ExternalInput")
    selt_in = nc.dram_tensor("selt", [NW * P, t_eff * P], BF16,
                             kind="ExternalInput")
    out = nc.dram_tensor("out", [NPC, NHF], F32, kind="ExternalOutput")

    tab_a = nc.dram_tensor("tab_a", [TABR, ROW], BF16)
    tab_b = nc.dram_tensor("tab_b", [TABR, ROW], BF16)

    with tile.TileContext(nc) as tc:
        with tc.tile_pool(name="const", bufs=1) as const:
            # --- consts and preloads ---
            wc = const.tile([P, 144], BF16)
            nc.sync.dma_start(wc[:], wcatb[:])
            b1 = const.tile([1, NHF], F32)
            nc.sync.dma_start(b1[:], bias_in[:])
            sbias = const.tile([P, NHF], F32)
            nc.gpsimd.partition_broadcast(sbias[:], b1[:])
            c_i32 = const.tile([P, P], mybir.dt.int32)
            nc.gpsimd.iota(c_i32[:], pattern=[[1, P]], base=0,
                           channel_multiplier=0)
            c_bf = const.tile([P, P], BF16)
            nc.vector.tensor_copy(c_bf[:], c_i32[:])
            bias_m24 = const.tile([P, 1], F32)
            nc.gpsimd.memset(bias_m24[:], -SHIFT)
            bias0 = const.tile([P, 1], F32)
            nc.gpsimd.memset(bias0[:], 0.0)
            padrow = const.tile([1, ROW], BF16)
            nc.gpsimd.memset(padrow[:], PADV)
            nc.sync.dma_start(tab_a[HALF:HALF + 1, :], padrow[:])
            nc.sync.dma_start(tab_b[HALF:HALF + 1, :], padrow[:])
            zmax = const.tile([P, t_eff * NH], F32)
            nc.gpsimd.memset(zmax[:], -1e30)
            stc = const.tile([P, NW * 8], BF16)  # [strg_hi(4)|strg_lo(4)]
            stc3 = stc[:].rearrange("p (w c) -> p w c", c=8)
            sWD = const.tile([P, NW * WEX], F32)  # [W(128)|D(4)] per window
            sWD3 = sWD[:].rearrange("p (w c) -> p w c", c=WEX)
            gidx_all = const.tile([P, NW * t_eff * 8], I16)
            nc.sync.dma_start(
                gidx_all[:].rearrange("p (w f) -> p w f", w=NW),
                gidx_in[:].rearrange("(w p) f -> p w f", p=P))
            gidx3 = gidx_all[:].rearrange("p (w f) -> p w f", w=NW)
            rel_all = const.tile([P, NW * t_eff], BF16)
            nc.sync.dma_start(
                rel_all[:].rearrange("p (w f) -> p w f", w=NW),
                rels_in[:].rearrange("(w p) f -> p w f", p=P))
            rel3 = rel_all[:].rearrange("p (w f) -> p w f", w=NW)

            # --- phase T: build gather tables from host-split xT hi/lo ---
            # ps cols 0:128 proj, 128:136 full fp32 scores [ssrc|strg] via
            # hi*W[0:144] + hi*wsa_lo + lo*wsa_hi into the 128:136 window.
            with tc.tile_pool(name="sbT", bufs=3) as sbT, \
                 tc.tile_pool(name="psT", bufs=2, space="PSUM") as psT:
                for sb_i in range(NPAD // 1024):
                    r0 = sb_i * 1024
                    xh = sbT.tile([P, 1024], BF16, tag="xh")
                    nc.sync.dma_start(xh[:], xTh[:, r0:r0 + 1024])
                    xl = sbT.tile([P, 1024], BF16, tag="xl")
                    nc.sync.dma_start(xl[:], xTl[:, r0:r0 + 1024])
                    tabt = sbT.tile([P, 8 * ROW], BF16, tag="tabt")
                    tb3 = tabt[:].rearrange("p (k f) -> p k f", f=ROW)
                    nc.gpsimd.memset(tb3[:, :, 144:256], 0.0)
                    for pair in range(4):
                        ps_p = psT.tile([P, 2 * 128], F32, tag="ps_p")
                        pp3 = ps_p[:].rearrange("p (i f) -> p i f", f=128)
                        ps_s = psT.tile([P, 2 * 8], F32, tag="ps_s")
                        ss3 = ps_s[:].rearrange("p (i f) -> p i f", f=8)
                        for i in range(2):
                            o = (pair * 2 + i) * 128
                            lhi = xh[:, o:o + 128]
                            llo = xl[:, o:o + 128]
                            nc.tensor.matmul(pp3[:, i, :], lhsT=lhi,
                                             rhs=wc[:, 0:128], start=True,
                                             stop=True)
                            nc.tensor.matmul(ss3[:, i, :], lhsT=lhi,
                                             rhs=wc[:, 128:136], start=True,
                                             stop=False)
                            nc.tensor.matmul(ss3[:, i, :], lhsT=lhi,
                                             rhs=wc[:, 136:144], start=False,
                                             stop=False)
                            nc.tensor.matmul(ss3[:, i, :], lhsT=llo,
                                             rhs=wc[:, 128:136], start=False,
                                             stop=True)
                        k = pair * 2
                        nc.scalar.copy(tb3[:, k:k + 2, 0:128], pp3[:])
                        # s slots: [128:136 hi | 136:144 lo], both [ssrc|strg]
                        nc.vector.tensor_copy(tb3[:, k:k + 2, 128:136], ss3)
                        slo = sbT.tile([P, 16], F32, tag="slo")
                        sl3 = slo[:].rearrange("p (i f) -> p i f", f=8)
                        nc.vector.tensor_tensor(sl3, ss3,
                                                tb3[:, k:k + 2, 128:136],
                                                OP.subtract)
                        nc.vector.tensor_copy(tb3[:, k:k + 2, 136:144], sl3)
                    if r0 + 1024 <= HALF:
                        nc.sync.dma_start(
                            tab_a[r0:r0 + 1024, :].rearrange(
                                "(k p) f -> p k f", p=P),
                            tb3[:, :, :])
                    elif r0 >= HALF:
                        nc.sync.dma_start(
                            tab_b[r0 - HALF:r0 - HALF + 1024, :].rearrange(
                                "(k p) f -> p k f", p=P),
                            tb3[:, :, :])
                    else:
                        nc.sync.dma_start(
                            tab_a[r0:HALF, :].rearrange(
                                "(k p) f -> p k f", p=P),
                            tb3[:, 0:4, :])
                        nc.sync.dma_start(
                            tab_b[0:r0 + 1024 - HALF, :].rearrange(
                                "(k p) f -> p k f", p=P),
                            tb3[:, 4:8, :])

                # --- stc: s_trg hi/lo for local targets from xlocT hi/lo ---
                for ci in range(7):
                    c0 = ci * 1024
                    clen = min(1024, NPC - c0)
                    xh = sbT.tile([P, 1024], BF16, tag="xh")
                    nc.sync.dma_start(xh[:, 0:clen], xlTh[:, c0:c0 + clen])
                    xl = sbT.tile([P, 1024], BF16, tag="xl")
                    nc.sync.dma_start(xl[:, 0:clen], xlTl[:, c0:c0 + clen])
                    for pair in range(max(1, clen // 256)):
                        nt2 = min(2, clen // 128 - pair * 2)
                        ps_c = psT.tile([P, 2 * 4], F32, tag="ps_c")
                        cc3 = ps_c[:].rearrange("p (i f) -> p i f", f=4)
                        for i in range(nt2):
                            o = (pair * 2 + i) * 128
                            lhi = xh[:, o:o + 128]
                            llo = xl[:, o:o + 128]
                            nc.tensor.matmul(cc3[:, i, :], lhsT=lhi,
                                             rhs=wc[:, 132:136], start=True,
                                             stop=False)
                            nc.tensor.matmul(cc3[:, i, :], lhsT=lhi,
                                             rhs=wc[:, 140:144], start=False,
                                             stop=False)
                            nc.tensor.matmul(cc3[:, i, :], lhsT=llo,
                                             rhs=wc[:, 132:136], start=False,
                                             stop=True)
                        w0 = c0 // P + pair * 2
                        nc.vector.tensor_copy(stc3[:, w0:w0 + nt2, 0:4],
                                              cc3[:, 0:nt2, :])
                        clo = sbT.tile([P, 8], F32, tag="clo")
                        cl3 = clo[:].rearrange("p (i f) -> p i f", f=4)
                        nc.vector.tensor_tensor(cl3[:, 0:nt2, :],
                                                cc3[:, 0:nt2, :],
                                                stc3[:, w0:w0 + nt2, 0:4],
                                                OP.subtract)
                        nc.vector.tensor_copy(stc3[:, w0:w0 + nt2, 4:8],
                                              cl3[:, 0:nt2, :])

            # --- phase E ---
            with tc.tile_pool(name="sbE", bufs=3) as sbE, \
                 tc.tile_pool(name="sbg", bufs=3) as sbg, \
                 tc.tile_pool(name="sbga", bufs=3) as sbga, \
                 tc.tile_pool(name="psE", bufs=3, space="PSUM") as psE:
                for w in range(NW):
                    gath = sbga.tile([P, t_eff * ROW], BF16, tag="gath")
                    g3 = gath[:].rearrange("p (t c) -> p t c", c=ROW)
                    nc.gpsimd.dma_gather(
                        out_ap=g3[:, 0:t_a, :], in_ap=tab_a[:],
                        idxs_ap=gidx3[:, w, 0:t_a * 8], num_idxs=t_a * P,
                        num_idxs_reg=t_a * P, elem_size=ROW,
                        single_packet=False)
                    nc.gpsimd.dma_gather(
                        out_ap=g3[:, t_a:t_eff, :], in_ap=tab_b[:],
                        idxs_ap=gidx3[:, w, t_a * 8:t_eff * 8],
                        num_idxs=t_b * P, num_idxs_reg=t_b * P, elem_size=ROW,
                        single_packet=False)
                    selt = sbg.tile([P, t_eff * P], BF16, tag="selt")
                    nc.sync.dma_start(selt[:], selt_in[w * P:(w + 1) * P, :])

                    # s_trg per edge via one-hot matmul
                    ps_st = psE.tile([P, t_eff * 8], F32, tag="ps_st")
                    st3 = ps_st[:].rearrange("p (t c) -> p t c", c=8)
                    for t in range(t_eff):
                        nc.tensor.matmul(st3[:, t, :],
                                         lhsT=selt[:, t * P:(t + 1) * P],
                                         rhs=stc3[:, w, :], start=True,
                                         stop=True)

                    # z = (ssrc_hi+strg_hi) + (ssrc_lo+strg_lo)
                    zs8 = sbE.tile([P, t_eff * 8], F32, tag="zs8")
                    z83 = zs8[:].rearrange("p (t h f) -> p t h f", h=2, f=4)
                    gsv = g3[:, :, 128:144].rearrange(
                        "p t (h g f) -> p t h g f", h=2, g=2)
                    nc.vector.tensor_tensor(
                        z83, gsv[:, :, :, 0, :],
                        st3.rearrange("p t (h f) -> p t h f", h=2), OP.add)
                    z = sbE.tile([P, t_eff * NH], F32, tag="z")
                    z3 = z[:].rearrange("p (t c) -> p t c", c=NH)
                    nc.vector.tensor_tensor(z3, z83[:, :, 0, :],
                                            z83[:, :, 1, :], OP.add)
                    nc.vector.tensor_tensor(zmax[:], zmax[:], z[:], OP.max)
                    # exp(lrelu(z)-24) == max(exp(0.2z-24), exp(z-24))
                    ea = sbE.tile([P, t_eff * NH], F32, tag="ea")
                    nc.scalar.activation(ea[:], z[:], ACT.Exp,
                                         bias=bias_m24[:], scale=LEAKY)
                    eb = sbE.tile([P, t_eff * NH], F32, tag="eb")
                    nc.scalar.activation(eb[:], z[:], ACT.Exp,
                                         bias=bias_m24[:])
                    wx = sbg.tile([P, t_eff * WEX], BF16, tag="wx")
                    wx3 = wx[:].rearrange("p (t c) -> p t c", c=WEX)
                    nc.vector.tensor_tensor(
                        wx3[:, :, 128:132],
                        ea[:].rearrange("p (t c) -> p t c", c=NH),
                        eb[:].rearrange("p (t c) -> p t c", c=NH), OP.max)

                    selb = sbg.tile([P, t_eff * P], BF16, tag="sel")
                    nc.vector.tensor_tensor(
                        selb[:].rearrange("p (t q) -> p t q", q=P),
                        rel3[:, w, :, None].to_broadcast([P, t_eff, P]),
                        c_bf[:, None, :].to_broadcast([P, t_eff, P]),
                        OP.is_equal)
                    sel = selb[:]
                    nc.vector.tensor_tensor(
                        wx3[:, :, 0:128].rearrange("p t (h f) -> p t h f",
                                                   f=FOUT),
                        g3[:, :, 0:128].rearrange("p t (h f) -> p t h f",
                                                  f=FOUT),
                        wx3[:, :, 128:132][:, :, :, None].to_broadcast(
                            [P, t_eff, NH, FOUT]),
                        OP.mult)

                    ps_o = psE.tile([P, WEX], F32, tag="ps_o")
                    for t in range(t_eff):
                        nc.tensor.matmul(ps_o[:],
                                         lhsT=sel[:, t * P:(t + 1) * P],
                                         rhs=wx3[:, t, :], start=(t == 0),
                                         stop=(t == t_eff - 1))
                    nc.scalar.copy(sWD3[:, w, :], ps_o[:])

                # --- global max + epsilon ---
                zm1 = sbE.tile([P, 1], F32, tag="zm1")
                nc.vector.tensor_reduce(zm1[:], zmax[:], axis=AX.X, op=OP.max)
                zma = sbE.tile([P, 1], F32, tag="zma")
                nc.gpsimd.partition_all_reduce(zma[:], zm1[:], channels=P,
                                               reduce_op=bass_isa.ReduceOp.max)
                with tc.tile_pool(name="dram", bufs=1, space="DRAM") as dram:
                    cc_in = dram.tile([1, 1], F32)
                    cc_out = dram.tile([1, 1], F32)
                    nc.sync.dma_start(cc_in[:], zma[0:1, :])
                    if sim_no_cc:
                        nc.sync.dma_start(cc_out[:], cc_in[:])
                    else:
                        nc.gpsimd.collective_compute(
                            "AllReduce", OP.max,
                            replica_groups=[list(range(N_CORES))],
                            ins=[cc_in.opt()], outs=[cc_out.opt()])
                    zg = sbE.tile([1, 1], F32, tag="zg")
                    nc.sync.dma_start(zg[:], cc_out[:])
                eg = sbE.tile([1, 1], F32, tag="eg")
                nc.vector.tensor_scalar_mul(eg[:], zg[:], LEAKY)
                nc.vector.tensor_tensor(eg[:], eg[:], zg[:], OP.max)
                ce = sbE.tile([1, 1], F32, tag="ce")
                nc.scalar.activation(ce[:], eg[:], ACT.Exp,
                                     bias=bias_m24[:1])
                nc.vector.tensor_scalar_mul(ce[:], ce[:], 1e-16)
                ceps = const.tile([P, 1], F32)
                nc.gpsimd.partition_broadcast(ceps[:], ce[:])
            # E pools closed here; phase F gets the freed SBUF
            with tc.tile_pool(name="sbFo", bufs=1) as sbFo:
                # --- phase F (xpb precomputed; 4 pipelined chunks) ---
                sbF = sbFo
                xw = sbF.tile([P, NW * NHF], F32, tag="xw")
                x3 = xw[:].rearrange("p (k f) -> p k f", f=NHF)
                nc.sync.dma_start(
                    x3, xloc[:].rearrange("(k p) f -> p k f", p=P))
                # x + bias does not depend on the collective; scheduled early
                nc.vector.tensor_tensor(
                    x3, x3, sbias[:, None, :].to_broadcast([P, NW, NHF]),
                    OP.add)
                den = sbF.tile([P, NW * NH], F32, tag="den")
                d3 = den[:].rearrange("p (k c) -> p k c", c=NH)
                nc.vector.tensor_tensor(
                    d3, sWD3[:, :, 128:132],
                    ceps[:, 0:1, None].to_broadcast([P, NW, NH]), OP.add)
                rec = sbF.tile([P, NW * NH], F32, tag="rec")
                nc.vector.reciprocal(rec[:], den[:])
                r3 = rec[:].rearrange("p (k c) -> p k c", c=NH)
                o1 = sbF.tile([P, NW * NHF], F32, tag="o1")
                o14 = o1[:].rearrange("p (k h f) -> p k h f", k=NW, h=NH)
                ee = sbF.tile([P, NW * NHF], F32, tag="ee")
                pos = sbF.tile([P, NW * NHF], F32, tag="pos")
                bounds = [0, 13, 25, 37, NW]
                for ci in range(4):
                    a, b = bounds[ci], bounds[ci + 1]
                    sl = slice(a * NHF, b * NHF)
                    nc.vector.tensor_tensor(
                        o14[:, a:b],
                        sWD3[:, a:b, 0:128].rearrange("p k (h f) -> p k h f",
                                                      f=FOUT),
                        r3[:, a:b, :, None].to_broadcast(
                            [P, b - a, NH, FOUT]),
                        OP.mult)
                    nc.vector.tensor_tensor(o1[:, sl], o1[:, sl], xw[:, sl],
                                            OP.add)
                    nc.scalar.activation(ee[:, sl], o1[:, sl], ACT.Exp,
                                         bias=bias0[:])
                    nc.scalar.activation(pos[:, sl], o1[:, sl], ACT.Relu,
                                         bias=bias0[:])
                    nc.vector.tensor_scalar(ee[:, sl], ee[:, sl], -1.0, 0.0,
                                            OP.add, OP.min)
                    nc.vector.tensor_tensor(ee[:, sl], ee[:, sl], pos[:, sl],
                                            OP.add)
                    nc.sync.dma_start(
                        out[a * P:b * P, :].rearrange("(k p) f -> p k f",
                                                      p=P),
                        ee[:].rearrange("p (k f) -> p k f", f=NHF)[:, a:b, :])

    nc.compile()
    return nc


def _make_inputs(x, edge_index, w_mat, a_src, a_trg, bias):
    t_a, t_b, gidx, rel_arr, selt = _prepare_edges(edge_index)
    x = np.ascontiguousarray(x, dtype=np.float32)
    xpad = np.zeros((NPAD, FIN), np.float32)
    xpad[:N_NODES] = x
    xT = np.ascontiguousarray(xpad.T)  # [128, 50176] f32
    xTh = xT.astype(BF)
    xTl = (xT - xTh.astype(np.float32)).astype(BF)

    asrc_m = np.zeros((NHF, NH), np.float32)
    atrg_m = np.zeros((NHF, NH), np.float32)
    for h in range(NH):
        asrc_m[h * FOUT:(h + 1) * FOUT, h] = a_src[h]
        atrg_m[h * FOUT:(h + 1) * FOUT, h] = a_trg[h]
    wsa = np.concatenate([w_mat @ asrc_m, w_mat @ atrg_m], axis=1)  # [128, 8]
    wsa_hi = wsa.astype(BF)
    wsa_lo = (wsa - wsa_hi.astype(np.float32)).astype(BF)
    wcatb = np.concatenate(
        [w_mat.astype(BF), wsa_hi, wsa_lo], axis=1)  # [128, 144]

    in_maps = []
    for c in range(N_CORES):
        in_maps.append({
            "xTh": xTh,
            "xTl": xTl,
            "xlTh": np.ascontiguousarray(xTh[:, c * NPC:(c + 1) * NPC]),
            "xlTl": np.ascontiguousarray(xTl[:, c * NPC:(c + 1) * NPC]),
            "xloc": np.ascontiguousarray(xpad[c * NPC:(c + 1) * NPC]),
            "wcatb": wcatb,
            "bias": np.ascontiguousarray(bias, dtype=np.float32).reshape(1, NHF),
            "gidx": np.ascontiguousarray(gidx[c]),
            "rels": np.ascontiguousarray(rel_arr[c]),
            "selt": np.ascontiguousarray(selt[c]),
        })
    return t_a, t_b, in_maps


def kernel(x, edge_index, W, a_src, a_trg, bias, _trace=False):
    from concourse.bass_utils import run_bass_kernel_spmd

    x = np.asarray(x)
    t_a, t_b, in_maps = _make_inputs(x, np.asarray(edge_index),
                                     np.asarray(W, dtype=np.float32),
                                     np.asarray(a_src, dtype=np.float32),
                                     np.asarray(a_trg, dtype=np.float32),
                                     np.asarray(bias, dtype=np.float32))
    nc = build_bass(t_a, t_b)
    res = run_bass_kernel_spmd(nc, in_maps, core_ids=list(range(N_CORES)),
                               trace=_trace)
    parts = []
    for c in range(N_CORES):
        valid = min(NPC, N_NODES - c * NPC)
        parts.append(res.results[c]["out"][:valid])
    out = np.concatenate(parts, axis=0)
    if _trace:
        kernel.last_results = res
    return out.astype(np.float32)


# revision 19
# speedup vs baseline: 1.2815x; 1.0224x over previous
# GAT (graph attention) layer on 8 Trainium2 NeuronCores — v2.
#
# Target-sharded edges (cores own 6272-aligned target ranges).  Per core:
#   Phase T: gather-table rows [proj(128)|ssrc_hi|strg_hi|ssrc_lo|strg_lo]
#     built from HOST-pretransposed, host-split bf16 hi/lo xT (no on-device
#     transposes or conversions); fp32-accurate scores via 3 hi/lo matmuls.
#     s_trg hi/lo for the core's own targets (stc) from xlocT, kept in SBUF.
#   Phase E: per 128-target window, two ucode dma_gathers fetch [proj|s] rows
#     by src; s_trg per edge via one-hot matmul with a host-streamed
#     transposed one-hot (selt); z -> leaky -> exp(z-24); one matmul per edge
#     tile aggregates [weighted-proj | denom] with targets on partitions
#     (PSUM accumulate, no transposes); per-window results stay in SBUF.
#   Collective: AllReduce(max) of one scalar (global score max M) reproduces
#     the reference's exp(e - e.max()) + 1e-16 epsilon numerics exactly.
#   Phase F: out = elu(W/(D + 1e-16*exp(M-24)) + x + bias); x+bias is
#     precomputed during phase E, the rest runs as 4 DVE/Act-pipelined chunks.
import sys

import numpy as np

sys.path.insert(0, "/opt/trn_rl_repo")

import ml_dtypes  # noqa: E402

import concourse.bass as bass  # noqa: E402,F401
import concourse.bass_isa as bass_isa  # noqa: E402
import concourse.mybir as mybir  # noqa: E402
import concourse.tile as tile  # noqa: E402
from concourse import bacc  # noqa: E402

P = 128
NH, FOUT = 4, 32
NHF = NH * FOUT  # 128
FIN = 128
ROW = 256  # bf16 elems per table row (512B); 144 used
WEX = NHF + NH  # 132: [weighted proj | ex]
LEAKY = 0.2
SHIFT = 24.0
N_NODES = 50000
N_CORES = 8
NPC = 6272  # 49 * 128, per-core padded target count
NW = 49
NPAD = 50176  # 98 * 512 = 49 * 1024, padded node count
HALF = 25088  # table split (A: [0, 25088), B: [25088, 50176))
TABR = HALF + 1  # +1 pad row (idx 25088) holding -1e4
PADV = -1e4
F32 = mybir.dt.float32
BF16 = mybir.dt.bfloat16
I16 = mybir.dt.int16
AX = mybir.AxisListType
OP = mybir.AluOpType
ACT = mybir.ActivationFunctionType
BF = ml_dtypes.bfloat16


def _wrap16(flat):
    """[..., L] -> dma_gather layout [..., 128, L//16] (16-wrap, replicated)."""
    L = flat.shape[-1]
    w = flat.reshape(flat.shape[:-1] + (L // 16, 16))
    w = np.swapaxes(w, -1, -2)
    return np.tile(w, (1, 1, 8, 1)).reshape(flat.shape[:-1] + (P, L // 16))


def _prepare_edges(edge_index):
    src = np.ascontiguousarray(edge_index[0]).astype(np.int64)
    trg = np.ascontiguousarray(edge_index[1]).astype(np.int64)
    E = src.shape[0]
    wglob = (trg // NPC) * NW + (trg % NPC) // P  # 0..391
    isb = (src >= HALF).astype(np.int64)
    order = np.argsort(wglob * 2 + isb, kind="stable")
    src_s, trg_s, wg_s, isb_s = src[order], trg[order], wglob[order], isb[order]
    nwin = N_CORES * NW
    cnt = np.bincount(wg_s * 2 + isb_s, minlength=2 * nwin)
    t_a = max(1, int(np.ceil(cnt[0::2].max() / P)))
    t_b = max(1, int(np.ceil(cnt[1::2].max() / P)))
    t_eff = t_a + t_b
    gkey = wg_s * 2 + isb_s
    gstart = np.concatenate([[0], np.cumsum(np.bincount(gkey, minlength=2 * nwin))])[:-1]
    jj = np.arange(E) - gstart[gkey]
    t_loc = jj // P
    p_idx = jj % P
    t_idx = np.where(isb_s == 1, t_a + t_loc, t_loc)
    c = wg_s // NW
    wloc = wg_s % NW
    rel = (trg_s % NPC) - wloc * P  # 0..127

    idx_a = np.full((N_CORES, NW, t_a * P), HALF, np.int16)  # pad row
    idx_b = np.full((N_CORES, NW, t_b * P), HALF, np.int16)
    ma = isb_s == 0
    idx_a[c[ma], wloc[ma], t_loc[ma] * P + p_idx[ma]] = src_s[ma].astype(np.int16)
    mb = isb_s == 1
    idx_b[c[mb], wloc[mb], t_loc[mb] * P + p_idx[mb]] = (src_s[mb] - HALF).astype(np.int16)

    rel_arr = np.full((N_CORES, NW * P, t_eff), -1.0, np.float32)
    rel_arr[c, wloc * P + p_idx, t_idx] = rel
    selt = np.zeros((N_CORES, NW * P, t_eff * P), BF)
    selt[c, wloc * P + rel, t_idx * P + p_idx] = 1.0

    ia = _wrap16(idx_a)
    ib = _wrap16(idx_b)
    gidx = np.concatenate([ia, ib], axis=-1).reshape(N_CORES, NW * P, t_eff * 8)
    return t_a, t_b, gidx, rel_arr.astype(BF), selt


def build_bass(t_a, t_b, sim_no_cc=False):
    t_eff = t_a + t_b
    nc = bacc.Bacc("TRN2", target_bir_lowering=False, debug=False,
                   num_devices=N_CORES)

    xTh = nc.dram_tensor("xTh", [P, NPAD], BF16, kind="ExternalInput")
    xTl = nc.dram_tensor("xTl", [P, NPAD], BF16, kind="ExternalInput")
    xlTh = nc.dram_tensor("xlTh", [P, NPC], BF16, kind="ExternalInput")
    xlTl = nc.dram_tensor("xlTl", [P, NPC], BF16, kind="ExternalInput")
    xloc = nc.dram_tensor("xloc", [NPC, FIN], F32, kind="ExternalInput")
    wcatb = nc.dram_tensor("wcatb", [P, 144], BF16, kind="ExternalInput")
    bias_in = nc.dram_tensor("bias", [1, NHF], F32, kind="ExternalInput")
    gidx_in = nc.dram_tensor("gidx", [NW * P, t_eff * 8], I16,
                             kind="ExternalInput")
    rels_in = nc.dram_tensor("rels", [NW * P, t_eff], BF16,
                             kind="ExternalInput")
    selt_in = nc.dram_tensor("selt", [NW * P, t_eff * P], BF16,
                             kind="ExternalInput")
    out = nc.dram_tensor("out", [NPC, NHF], F32, kind="ExternalOutput")

    tab_a = nc.dram_tensor("tab_a", [TABR, ROW], BF16)
    tab_b = nc.dram_tensor("tab_b", [TABR, ROW], BF16)

    with tile.TileContext(nc) as tc:
        with tc.tile_pool(name="const", bufs=1) as const:
            # --- consts and preloads ---
            wc = const.tile([P, 144], BF16)
            nc.sync.dma_start(wc[:], wcatb[:])
            b1 = const.tile([1, NHF], F32)
            nc.sync.dma_start(b1[:], bias_in[:])
            sbias = const.tile([P, NHF], F32)
            nc.gpsimd.partition_broadcast(sbias[:], b1[:])
            c_i32 = const.tile([P, P], mybir.dt.int32)
            nc.gpsimd.iota(c_i32[:], pattern=[[1, P]], base=0,
                           channel_multiplier=0)
            c_bf = const.tile([P, P], BF16)
            nc.vector.tensor_copy(c_bf[:], c_i32[:])
            bias_m24 = const.tile([P, 1], F32)
            nc.gpsimd.memset(bias_m24[:], -SHIFT)
            bias0 = const.tile([P, 1], F32)
            nc.gpsimd.memset(bias0[:], 0.0)
            padrow = const.tile([1, ROW], BF16)
            nc.gpsimd.memset(padrow[:], PADV)
            nc.sync.dma_start(tab_a[HALF:HALF + 1, :], padrow[:])
            nc.sync.dma_start(tab_b[HALF:HALF + 1, :], padrow[:])
            zmax = const.tile([P, t_eff * NH], F32)
            nc.gpsimd.memset(zmax[:], -1e30)
            stc = const.tile([P, NW * 8], BF16)  # [strg_hi(4)|strg_lo(4)]
            stc3 = stc[:].rearrange("p (w c) -> p w c", c=8)
            sWD = const.tile([P, NW * WEX], F32)  # [W(128)|D(4)] per window
            sWD3 = sWD[:].rearrange("p (w c) -> p w c", c=WEX)
            gidx_all = const.tile([P, NW * t_eff * 8], I16)
            nc.sync.dma_start(
                gidx_all[:].rearrange("p (w f) -> p w f", w=NW),
                gidx_in[:].rearrange("(w p) f -> p w f", p=P))
            gidx3 = gidx_all[:].rearrange("p (w f) -> p w f", w=NW)
            rel_all = const.tile([P, NW * t_eff], BF16)
            nc.sync.dma_start(
                rel_all[:].rearrange("p (w f) -> p w f", w=NW),
                rels_in[:].rearrange("(w p) f -> p w f", p=P))
            rel3 = rel_all[:].rearrange("p (w f) -> p w f", w=NW)

            # --- phase T: build gather tables from host-split xT hi/lo ---
            # ps cols 0:128 proj, 128:136 full fp32 scores [ssrc|strg] via
            # hi*W[0:144] + hi*wsa_lo + lo*wsa_hi into the 128:136 window.
            with tc.tile_pool(name="sbT", bufs=4) as sbT, \
                 tc.tile_pool(name="psT", bufs=2, space="PSUM") as psT:
                for sb_i in range(NPAD // 1024):
                    r0 = sb_i * 1024
                    xh = sbT.tile([P, 1024], BF16, tag="xh")
                    nc.sync.dma_start(xh[:], xTh[:, r0:r0 + 1024])
                    xl = sbT.tile([P, 1024], BF16, tag="xl")
                    nc.sync.dma_start(xl[:], xTl[:, r0:r0 + 1024])
                    tabt = sbT.tile([P, 8 * ROW], BF16, tag="tabt")
                    tb3 = tabt[:].rearrange("p (k f) -> p k f", f=ROW)
                    nc.gpsimd.memset(tb3[:, :, 144:256], 0.0)
                    for pair in range(4):
                        ps_p = psT.tile([P, 2 * 128], F32, tag="ps_p")
                        pp3 = ps_p[:].rearrange("p (i f) -> p i f", f=128)
                        ps_s = psT.tile([P, 2 * 8], F32, tag="ps_s")
                        ss3 = ps_s[:].rearrange("p (i f) -> p i f", f=8)
                        for i in range(2):
                            o = (pair * 2 + i) * 128
                            lhi = xh[:, o:o + 128]
                            llo = xl[:, o:o + 128]
                            nc.tensor.matmul(pp3[:, i, :], lhsT=lhi,
                                             rhs=wc[:, 0:128], start=True,
                                             stop=True)
                            nc.tensor.matmul(ss3[:, i, :], lhsT=lhi,
                                             rhs=wc[:, 128:136], start=True,
                                             stop=False)
                            nc.tensor.matmul(ss3[:, i, :], lhsT=lhi,
                                             rhs=wc[:, 136:144], start=False,
                                             stop=False)
                            nc.tensor.matmul(ss3[:, i, :], lhsT=llo,
                                             rhs=wc[:, 128:136], start=False,
                                             stop=True)
                        k = pair * 2
                        nc.scalar.copy(tb3[:, k:k + 2, 0:128], pp3[:])
                        # s slots: [128:136 hi | 136:144 lo], both [ssrc|strg]
                        nc.vector.tensor_copy(tb3[:, k:k + 2, 128:136], ss3)
                        slo = sbT.tile([P, 16], F32, tag="slo")
                        sl3 = slo[:].rearrange("p (i f) -> p i f", f=8)
                        nc.vector.tensor_tensor(sl3, ss3,
                                                tb3[:, k:k + 2, 128:136],
                                                OP.subtract)
                        nc.vector.tensor_copy(tb3[:, k:k + 2, 136:144], sl3)
                    if r0 + 1024 <= HALF:
                        nc.sync.dma_start(
                            tab_a[r0:r0 + 1024, :].rearrange(
                                "(k p) f -> p k f", p=P),
                            tb3[:, :, :])
                    elif r0 >= HALF:
                        nc.sync.dma_start(
                            tab_b[r0 - HALF:r0 - HALF + 1024, :].rearrange(
                                "(k p) f -> p k f", p=P),
                            tb3[:, :, :])
                    else:
                        nc.sync.dma_start(
                            tab_a[r0:HALF, :].rearrange(
                                "(k p) f -> p k f", p=P),
                            tb3[:, 0:4, :])
                        nc.sync.dma_start(
                            tab_b[0:r0 + 1024 - HALF, :].rearrange(
                                "(k p) f -> p k f", p=P),
                            tb3[:, 4:8, :])

                # --- stc: s_trg hi/lo for local targets from xlocT hi/lo ---
                for ci in range(7):
                    c0 = ci * 1024
                    clen = min(1024, NPC - c0)
                    xh = sbT.tile([P, 1024], BF16, tag="xh")
                    nc.sync.dma_start(xh[:, 0:clen], xlTh[:, c0:c0 + clen])
                    xl = sbT.tile([P, 1024], BF16, tag="xl")
                    nc.sync.dma_start(xl[:, 0:clen], xlTl[:, c0:c0 + clen])
                    for pair in range(max(1, clen // 256)):
                        nt2 = min(2, clen // 128 - pair * 2)
                        ps_c = psT.tile([P, 2 * 4], F32, tag="ps_c")
                        cc3 = ps_c[:].rearrange("p (i f) -> p i f", f=4)
                        for i in range(nt2):
                            o = (pair * 2 + i) * 128
                            lhi = xh[:, o:o + 128]
                            llo = xl[:, o:o + 128]
                            nc.tensor.matmul(cc3[:, i, :], lhsT=lhi,
                                             rhs=wc[:, 132:136], start=True,
                                             stop=False)
                            nc.tensor.matmul(cc3[:, i, :], lhsT=lhi,
                                             rhs=wc[:, 140:144], start=False,
                                             stop=False)
                            nc.tensor.matmul(cc3[:, i, :], lhsT=llo,
                                             rhs=wc[:, 132:136], start=False,
                                             stop=True)
                        w0 = c0 // P + pair * 2
                        nc.vector.tensor_copy(stc3[:, w0:w0 + nt2, 0:4],
                                              cc3[:, 0:nt2, :])
                        clo = sbT.tile([P, 8], F32, tag="clo")
                        cl3 = clo[:].rearrange("p (i f) -> p i f", f=4)
                        nc.vector.tensor_tensor(cl3[:, 0:nt2, :],
                                                cc3[:, 0:nt2, :],
                                                stc3[:, w0:w0 + nt2, 0:4],
                                                OP.subtract)
                        nc.vector.tensor_copy(stc3[:, w0:w0 + nt2, 4:8],
                                              cl3[:, 0:nt2, :])

            # --- phase E ---
            with tc.tile_pool(name="sbE", bufs=3) as sbE, \
                 tc.tile_pool(name="sbg", bufs=3) as sbg, \
                 tc.tile_pool(name="sbga", bufs=3) as sbga, \
                 tc.tile_pool(name="psE", bufs=3, space="PSUM") as psE:
                for w in range(NW):
                    gath = sbga.tile([P, t_eff * ROW], BF16, tag="gath")
                    g3 = gath[:].rearrange("p (t c) -> p t c", c=ROW)
                    nc.gpsimd.dma_gather(
                        out_ap=g3[:, 0:t_a, :], in_ap=tab_a[:],
                        idxs_ap=gidx3[:, w, 0:t_a * 8], num_idxs=t_a * P,
                        num_idxs_reg=t_a * P, elem_size=ROW,
                        single_packet=False)
                    nc.gpsimd.dma_gather(
                        out_ap=g3[:, t_a:t_eff, :], in_ap=tab_b[:],
                        idxs_ap=gidx3[:, w, t_a * 8:t_eff * 8],
                        num_idxs=t_b * P, num_idxs_reg=t_b * P, elem_size=ROW,
                        single_packet=False)
                    selt = sbg.tile([P, t_eff * P], BF16, tag="selt")
                    nc.sync.dma_start(selt[:], selt_in[w * P:(w + 1) * P, :])

                    # s_trg per edge via one-hot matmul
                    ps_st = psE.tile([P, t_eff * 8], F32, tag="ps_st")
                    st3 = ps_st[:].rearrange("p (t c) -> p t c", c=8)
                    for t in range(t_eff):
                        nc.tensor.matmul(st3[:, t, :],
                                         lhsT=selt[:, t * P:(t + 1) * P],
                                         rhs=stc3[:, w, :], start=True,
                                         stop=True)

                    # z = (ssrc_hi+strg_hi) + (ssrc_lo+strg_lo)
                    zs8 = sbE.tile([P, t_eff * 8], F32, tag="zs8")
                    z83 = zs8[:].rearrange("p (t h f) -> p t h f", h=2, f=4)
                    gsv = g3[:, :, 128:144].rearrange(
                        "p t (h g f) -> p t h g f", h=2, g=2)
                    nc.vector.tensor_tensor(
                        z83, gsv[:, :, :, 0, :],
                        st3.rearrange("p t (h f) -> p t h f", h=2), OP.add)
                    z = sbE.tile([P, t_eff * NH], F32, tag="z")
                    z3 = z[:].rearrange("p (t c) -> p t c", c=NH)
                    nc.vector.tensor_tensor(z3, z83[:, :, 0, :],
                                            z83[:, :, 1, :], OP.add)
                    nc.vector.tensor_tensor(zmax[:], zmax[:], z[:], OP.max)
                    # exp(lrelu(z)-24) == max(exp(0.2z-24), exp(z-24))
                    ea = sbE.tile([P, t_eff * NH], F32, tag="ea")
                    nc.scalar.activation(ea[:], z[:], ACT.Exp,
                                         bias=bias_m24[:], scale=LEAKY)
                    eb = sbE.tile([P, t_eff * NH], F32, tag="eb")
                    nc.scalar.activation(eb[:], z[:], ACT.Exp,
                                         bias=bias_m24[:])
                    wx = sbg.tile([P, t_eff * WEX], BF16, tag="wx")
                    wx3 = wx[:].rearrange("p (t c) -> p t c", c=WEX)
                    nc.vector.tensor_tensor(
                        wx3[:, :, 128:132],
                        ea[:].rearrange("p (t c) -> p t c", c=NH),
                        eb[:].rearrange("p (t c) -> p t c", c=NH), OP.max)

                    selb = sbg.tile([P, t_eff * P], BF16, tag="sel")
                    nc.vector.tensor_tensor(
                        selb[:].rearrange("p (t q) -> p t q", q=P),
                        rel3[:, w, :, None].to_broadcast([P, t_eff, P]),
                        c_bf[:, None, :].to_broadcast([P, t_eff, P]),
                        OP.is_equal)
                    sel = selb[:]
                    nc.vector.tensor_tensor(
                        wx3[:, :, 0:128].rearrange("p t (h f) -> p t h f",
                                                   f=FOUT),
                        g3[:, :, 0:128].rearrange("p t (h f) -> p t h f",
                                                  f=FOUT),
                        wx3[:, :, 128:132][:, :, :, None].to_broadcast(
                            [P, t_eff, NH, FOUT]),
                        OP.mult)

                    ps_o = psE.tile([P, WEX], F32, tag="ps_o")
                    for t in range(t_eff):
                        nc.tensor.matmul(ps_o[:],
                                         lhsT=sel[:, t * P:(t + 1) * P],
                                         rhs=wx3[:, t, :], start=(t == 0),
                                         stop=(t == t_eff - 1))
                    nc.scalar.copy(sWD3[:, w, :], ps_o[:])

                # --- global max + epsilon ---
                zm1 = sbE.tile([P, 1], F32, tag="zm1")
                nc.vector.tensor_reduce(zm1[:], zmax[:], axis=AX.X, op=OP.max)
                zma = sbE.tile([P, 1], F32, tag="zma")
                nc.gpsimd.partition_all_reduce(zma[:], zm1[:], channels=P,
                                               reduce_op=bass_isa.ReduceOp.max)
                with tc.tile_pool(name="dram", bufs=1, space="DRAM") as dram:
                    cc_in = dram.tile([1, 1], F32)
                    cc_out = dram.tile([1, 1], F32)
                    nc.sync.dma_start(cc_in[:], zma[0:1, :])
                    if sim_no_cc:
                        nc.sync.dma_start(cc_out[:], cc_in[:])
                    else:
                        nc.gpsimd.collective_compute(
                            "AllReduce", OP.max,
                            replica_groups=[list(range(N_CORES))],
                            ins=[cc_in.opt()], outs=[cc_out.opt()])
                    zg = sbE.tile([1, 1], F32, tag="zg")
                    nc.sync.dma_start(zg[:], cc_out[:])
                eg = sbE.tile([1, 1], F32, tag="eg")
                nc.vector.tensor_scalar_mul(eg[:], zg[:], LEAKY)
                nc.vector.tensor_tensor(eg[:], eg[:], zg[:], OP.max)
                ce = sbE.tile([1, 1], F32, tag="ce")
                nc.scalar.activation(ce[:], eg[:], ACT.Exp,
                                     bias=bias_m24[:1])
                nc.vector.tensor_scalar_mul(ce[:], ce[:], 1e-16)
                ceps = const.tile([P, 1], F32)
                nc.gpsimd.partition_broadcast(ceps[:], ce[:])
            # E pools closed here; phase F gets the freed SBUF
            with tc.tile_pool(name="sbFo", bufs=1) as sbFo:
                # --- phase F (xpb precomputed; 4 pipelined chunks) ---
                sbF = sbFo
                xw = sbF.tile([P, NW * NHF], F32, tag="xw")
                x3 = xw[:].rearrange("p (k f) -> p k f", f=NHF)
                nc.sync.dma_start(
                    x3, xloc[:].rearrange("(k p) f -> p k f", p=P))
                # x + bias does not depend on the collective; scheduled early
                nc.vector.tensor_tensor(
                    x3, x3, sbias[:, None, :].to_broadcast([P, NW, NHF]),
                    OP.add)
                den = sbF.tile([P, NW * NH], F32, tag="den")
                d3 = den[:].rearrange("p (k c) -> p k c", c=NH)
                nc.vector.tensor_tensor(
                    d3, sWD3[:, :, 128:132],
                    ceps[:, 0:1, None].to_broadcast([P, NW, NH]), OP.add)
                rec = sbF.tile([P, NW * NH], F32, tag="rec")
                nc.vector.reciprocal(rec[:], den[:])
                r3 = rec[:].rearrange("p (k c) -> p k c", c=NH)
                o1 = sbF.tile([P, NW * NHF], F32, tag="o1")
                o14 = o1[:].rearrange("p (k h f) -> p k h f", k=NW, h=NH)
                ee = sbF.tile([P, NW * NHF], F32, tag="ee")
                pos = sbF.tile([P, NW * NHF], F32, tag="pos")
                bounds = [0, 13, 25, 37, NW]
                for ci in range(4):
                    a, b = bounds[ci], bounds[ci + 1]
                    sl = slice(a * NHF, b * NHF)
                    nc.vector.tensor_tensor(
                        o14[:, a:b],
                        sWD3[:, a:b, 0:128].rearrange("p k (h f) -> p k h f",
                                                      f=FOUT),
                        r3[:, a:b, :, None].to_broadcast(
                            [P, b - a, NH, FOUT]),
                        OP.mult)
                    nc.vector.tensor_tensor(o1[:, sl], o1[:, sl], xw[:, sl],
                                            OP.add)
                    nc.scalar.activation(ee[:, sl], o1[:, sl], ACT.Exp,
                                         bias=bias0[:])
                    nc.scalar.activation(pos[:, sl], o1[:, sl], ACT.Relu,
                                         bias=bias0[:])
                    nc.vector.tensor_scalar(ee[:, sl], ee[:, sl], -1.0, 0.0,
                                            OP.add, OP.min)
                    nc.vector.tensor_tensor(ee[:, sl], ee[:, sl], pos[:, sl],
                                            OP.add)
                    nc.sync.dma_start(
                        out[a * P:b * P, :].rearrange("(k p) f -> p k f",
                                                      p=P),
                        ee[:].rearrange("p (k f) -> p k f", f=NHF)[:, a:b, :])

    nc.compile()
    return nc


def _make_inputs(x, edge_index, w_mat, a_src, a_trg, bias):
    t_a, t_b, gidx, rel_arr, selt = _prepare_edges(edge_index)
    x = np.ascontiguousarray(x, dtype=np.float32)
    xpad = np.zeros((NPAD, FIN), np.float32)
    xpad[:N_NODES] = x
    xT = np.ascontiguousarray(xpad.T)  # [128, 50176] f32
    xTh = xT.astype(BF)
    xTl = (xT - xTh.astype(np.float32)).astype(BF)

    asrc_m = np.zeros((NHF, NH), np.float32)
    atrg_m = np.zeros((NHF, NH), np.float32)
    for h in range(NH):
        asrc_m[h * FOUT:(h + 1) * FOUT, h] = a_src[h]
        atrg_m[h * FOUT:(h + 1) * FOUT, h] = a_trg[h]
    wsa = np.concatenate([w_mat @ asrc_m, w_mat @ atrg_m], axis=1)  # [128, 8]
    wsa_hi = wsa.astype(BF)
    wsa_lo = (wsa - wsa_hi.astype(np.float32)).astype(BF)
    wcatb = np.concatenate(
        [w_mat.astype(BF), wsa_hi, wsa_lo], axis=1)  # [128, 144]

    in_maps = []
    for c in range(N_CORES):
        in_maps.append({
            "xTh": xTh,
            "xTl": xTl,
            "xlTh": np.ascontiguousarray(xTh[:, c * NPC:(c + 1) * NPC]),
            "xlTl": np.ascontiguousarray(xTl[:, c * NPC:(c + 1) * NPC]),
            "xloc": np.ascontiguousarray(xpad[c * NPC:(c + 1) * NPC]),
            "wcatb": wcatb,
            "bias": np.ascontiguousarray(bias, dtype=np.float32).reshape(1, NHF),
            "gidx": np.ascontiguousarray(gidx[c]),
            "rels": np.ascontiguousarray(rel_arr[c]),
            "selt": np.ascontiguousarray(selt[c]),
        })
    return t_a, t_b, in_maps


def kernel(x, edge_index, W, a_src, a_trg, bias, _trace=False):
    from concourse.bass_utils import run_bass_kernel_spmd

    x = np.asarray(x)
    t_a, t_b, in_maps = _make_inputs(x, np.asarray(edge_index),
                                     np.asarray(W, dtype=np.float32),
                                     np.asarray(a_src, dtype=np.float32),
                                     np.asarray(a_trg, dtype=np.float32),
                                     np.asarray(bias, dtype=np.float32))
    nc = build_bass(t_a, t_b)
    res = run_bass_kernel_spmd(nc, in_maps, core_ids=list(range(N_CORES)),
                               trace=_trace)
    parts = []
    for c in range(N_CORES):
        valid = min(NPC, N_NODES - c * NPC)
        parts.append(res.results[c]["out"][:valid])
    out = np.concatenate(parts, axis=0)
    if _trace:
        kernel.last_results = res
    return out.astype(np.float32)
